# revision 20
# baseline (speedup 1.0000x reference)
"""Trainium2 Bass kernel for nn_AdaptiveLNN (2x LTC recurrent layers + MHA + head).

Strategy:
  - Pure data parallelism: B=64 sharded over 8 NeuronCores (Bc=8/core), zero
    collectives. Each core runs the full scan for its shard.
  - Transposed layout on chip: activations stored (128 part = h%128,
    free = (h_chunk, b)) -> tiles (128, 4, 8).
  - Input-dependent matmuls (x @ Win.T, x-part of tau_a) precomputed in bulk.
  - LayerNorm never materialized: all consumers are linear, so LN folds into
    the following matmul (host folds ln_w/ln_b into weights; m/rstd folded on
    chip:  LN(h) @ W'.T = rstd*(h @ W'.T - m*rowsum(W')) + bias').
  - Attention: reference uses only the LAST timestep of attention output, so
    only q[T-1] is needed -> O(T) attention.
  - The program is split into 3 sequential TileContexts (P1+scan0 | P3+scan1 |
    attention) so per-semaphore increment counts stay under the HW ceiling;
    contexts hand data across via raw DRAM tensors (ordered by the all-engine
    barrier at each TileContext exit).
"""

import numpy as np

B, T, IN, H, OUT, NH = 64, 512, 256, 512, 256, 4
HD = H // NH
DT = 0.1
UNFOLDS = 6
MIN_TAU, MAX_TAU = 0.1, 10.0
NCORES = 8
BC = B // NCORES          # 8
HC = H // 128             # 4
P = 128
FW = HC * BC              # 32
EPS = 1e-5

_CACHE = {}


# ---------------------------------------------------------------- host packing

def _wT(Wt):
    """(out_f, in_f) -> lhsT sbuf layout (128, nk*out_f):
    [p, kc*out_f + m] = W[m, kc*128 + p]."""
    Wt = np.ascontiguousarray(Wt, np.float32)
    of, inf_ = Wt.shape
    nk = inf_ // P
    a = Wt.T.reshape(nk, P, of)
    return np.ascontiguousarray(a.transpose(1, 0, 2).reshape(P, nk * of))


def _bcast(vec):
    """(H,) -> (128, HC, BC): [p, hc, b] = vec[hc*128+p]."""
    a = np.asarray(vec, np.float32).reshape(HC, P).T
    return np.ascontiguousarray(
        np.repeat(a[:, :, None], BC, axis=2).reshape(P, HC, BC))


def _perH(vec):
    """(F,) -> (128, F//128): [p, c] = vec[c*128+p]."""
    v = np.asarray(vec, np.float32)
    return np.ascontiguousarray(v.reshape(v.size // P, P).T)


def _xT(x):
    """(Bc, Tn, F) -> (128, F//128, Tn*Bc): [p, kc, t*Bc+b] = x[b, t, kc*128+p]."""
    Bc, Tn, F = x.shape
    nk = F // P
    a = x.transpose(2, 1, 0).reshape(nk, P, Tn, Bc)
    return np.ascontiguousarray(
        a.transpose(1, 0, 2, 3).reshape(P, nk, Tn * Bc).astype(np.float32))


def _fold3(Wt, bias, ln_w, ln_b):
    """Fold input-LN affine into weight/bias; return (W', bias', rowsum(W'))."""
    Wt = np.asarray(Wt, np.float32)
    Wp = Wt * np.asarray(ln_w, np.float32)[None, :]
    bp = np.asarray(bias, np.float32) + Wt @ np.asarray(ln_b, np.float32)
    return Wp, bp, Wp.sum(axis=1)


# ---------------------------------------------------------------- builder

def _build(Tn=T, taps=()):
    import concourse.bass as bass
    import concourse.mybir as mybir
    from concourse import bacc
    from concourse.tile import TileContext
    from concourse.masks import make_identity

    f32 = mybir.dt.float32
    ALU = mybir.AluOpType
    ACTF = mybir.ActivationFunctionType

    NB = max(1, (Tn * BC) // 512)      # bulk N-chunks over (t, b)
    NBW = (Tn * BC) // NB              # bulk N width (<= 512)
    TB = NBW // BC                     # timesteps per bulk chunk
    NT = Tn // P                       # t-chunks of 128
    CH = 16                            # scan stream chunk (steps)

    nc = bacc.Bacc("TRN2", target_bir_lowering=False)

    def par(name, shape):
        return nc.declare_dram_parameter(name, list(shape), f32, isOutput=False)

    PARAMS = [
        ("x_T", (P, IN // P, Tn * BC)),
        ("win0T", (P, (IN // P) * H)), ("tau0axT", (P, (IN // P) * H)),
        ("tau0avT", (P, HC * H)), ("tau0bT", (P, HC * H)), ("wrec0T", (P, HC * H)),
        ("win1T", (P, HC * H)), ("tau1axT", (P, HC * H)),
        ("tau1avT", (P, HC * H)), ("tau1bT", (P, HC * H)), ("wrec1T", (P, HC * H)),
        ("wkT", (P, HC * H)), ("wvT", (P, HC * H)), ("wqT", (P, HC * H)),
        ("woT", (P, HC * H)), ("p1T", (P, HC * (H // 2))), ("p2T", (P, 2 * OUT)),
        ("b_i0", (P, HC)), ("b_x0", (P, HC)),
        ("b_i1", (P, HC)), ("b_x1", (P, HC)),
        ("b_k", (P, HC)), ("b_q", (P, HC)),
        ("b_o", (P, HC)), ("b_p1", (P, 2)), ("b_p2", (P, 2)),
        ("nrs_i1", (P, HC)), ("nrs_x1", (P, HC)), ("nrs_k", (P, HC)),
        ("gs0bc", (P, HC, BC)), ("gl0bc", (P, HC, BC)), ("tbb0bc", (P, HC, BC)),
        ("gs1bc", (P, HC, BC)), ("gl1bc", (P, HC, BC)), ("tbb1bc", (P, HC, BC)),
        ("rsv_flat", (P, H)), ("bv_flat", (P, H)),
    ]
    PR = {name: par(name, shape) for name, shape in PARAMS}
    out_p = nc.declare_dram_parameter("out", [BC, OUT], f32, isOutput=True)
    tap_h = {}
    SMALL_TAPS = {
        "q": [P, HC, BC], "sc": [NH * BC, Tn], "en": [NH * BC, Tn],
        "oT": [P, HC, BC], "ao": [P, HC, BC], "Vb0": [P, NT * H],
        "KTc": [P, Tn * BC], "xh": [P, HC, BC],
    }
    stp = {}
    for tname in taps:
        if tname in SMALL_TAPS:
            stp[tname] = nc.declare_dram_parameter(
                "tap_" + tname, SMALL_TAPS[tname], f32, isOutput=True)
        else:
            tap_h[tname] = nc.declare_dram_parameter(
                "tap_" + tname, [P, Tn, HC, BC], f32, isOutput=True)

    # Cross-context intermediates (ordered by TileContext exit barriers).
    h0B = nc.dram_tensor("h0B", [P, Tn, HC, BC], f32)
    h1B = nc.dram_tensor("h1B", [P, Tn, HC, BC], f32)
    h1D = nc.dram_tensor("h1D", [P, BC, Tn, HC], f32)

    # ---------------- shared helpers ----------------
    def load(pool, *names):
        out = {}
        for nm in names:
            t_ = pool.tile(list(PR[nm].shape), f32, tag=nm, name=nm)
            nc.sync.dma_start(out=t_[:], in_=PR[nm][:])
            out[nm] = t_
        return out

    def mmT(ps, w_sb, rhs, nk, hcs=HC, wof=H):
        for hc in range(hcs):
            for kc in range(nk):
                nc.tensor.matmul(
                    ps[:, hc],
                    w_sb[:, kc * wof + hc * P: kc * wof + hc * P + P],
                    rhs[:, kc],
                    start=(kc == 0), stop=(kc == nk - 1))

    def consts(cp):
        ones_col = cp.tile([P, 1], f32, name="ones_col")
        nc.vector.memset(ones_col[:], 1.0)
        ones_row = cp.tile([1, P], f32, name="ones_row")
        nc.vector.memset(ones_row[:], 1.0)
        eps_c = cp.tile([1, 1], f32, name="eps_c")
        nc.vector.memset(eps_c[:], EPS)
        return ones_col, ones_row, eps_c

    def stap(name, ap):
        if name in stp:
            nc.sync.dma_start(out=stp[name][tuple(
                slice(0, s) for s in stp[name].shape)], in_=ap)

    def copy_tap(tc, src, name):
        if name in tap_h:
            with tc.tile_pool(name="tap" + name, bufs=2) as tpp:
                for t in range(Tn):
                    tt = tpp.tile([P, HC, BC], f32, tag="t", name="tt")
                    nc.sync.dma_start(out=tt[:], in_=src[:, t])
                    nc.sync.dma_start(out=tap_h[name][:, t], in_=tt[:])

    def scan_layer(tc, layer, sw, v, g, Isrc, Xsrc, hout, houtD=None):
        L = str(layer)
        wrec, tauav, taub = sw["wrec" + L + "T"], sw["tau" + L + "avT"], sw["tau" + L + "bT"]
        gsbc, glbc, tbbbc = sw["gs" + L + "bc"], sw["gl" + L + "bc"], sw["tbb" + L + "bc"]
        with tc.tile_pool(name="scps" + L, bufs=2, space="PSUM") as pps, \
             tc.tile_pool(name="scwk" + L, bufs=3) as wk, \
             tc.tile_pool(name="scst" + L, bufs=2) as sst:
            ich = xch = None
            for t in range(Tn):
                if t % CH == 0:
                    ich = sst.tile([P, CH, HC, BC], f32, tag="ich", name="ich")
                    xch = sst.tile([P, CH, HC, BC], f32, tag="xch", name="xch")
                    nc.sync.dma_start(out=ich[:], in_=Isrc[:, t:t + CH])
                    nc.sync.dma_start(out=xch[:], in_=Xsrc[:, t:t + CH])
                I0t = ich[:, t % CH]
                X0t = xch[:, t % CH]

                # tau net (uses v at step start)
                psA = pps.tile([P, HC, BC], f32, tag="psA", name="psA")
                mmT(psA, tauav, v, HC)
                u2 = wk.tile([P, HC, BC], f32, tag="u2", name="u2")
                nc.vector.tensor_add(u2[:], psA[:], X0t)
                th2 = wk.tile([P, HC, BC], f32, tag="th2", name="th2")
                nc.scalar.activation(th2[:], u2[:], ACTF.Tanh)
                psB = pps.tile([P, HC, BC], f32, tag="psB", name="psB")
                mmT(psB, taub, th2, HC)
                u3 = wk.tile([P, HC, BC], f32, tag="u3", name="u3")
                nc.vector.tensor_add(u3[:], psB[:], tbbbc[:])
                sig = wk.tile([P, HC, BC], f32, tag="sig", name="sig")
                nc.scalar.activation(sig[:], u3[:], ACTF.Sigmoid)
                tau = wk.tile([P, HC, BC], f32, tag="tau", name="tau")
                nc.vector.tensor_scalar(tau[:], sig[:], MAX_TAU - MIN_TAU,
                                        MIN_TAU, op0=ALU.mult, op1=ALU.add)
                rtau = wk.tile([P, HC, BC], f32, tag="rtau", name="rtau")
                nc.vector.reciprocal_approx_fast(out=rtau[:], in_=tau[:])
                kap = wk.tile([P, HC, BC], f32, tag="kap", name="kap")
                nc.vector.tensor_scalar(kap[:], rtau[:], DT / 0.5, None,
                                        op0=ALU.mult)
                gam = wk.tile([P, HC, BC], f32, tag="gam", name="gam")
                nc.vector.tensor_mul(gam[:], rtau[:], gsbc[:])  # gs pre-scaled DT
                tl = wk.tile([P, HC, BC], f32, tag="tl", name="tl")
                nc.vector.tensor_mul(tl[:], rtau[:], glbc[:])   # gl pre-scaled DT
                cL = wk.tile([P, HC, BC], f32, tag="cL", name="cL")
                nc.vector.tensor_scalar(cL[:], tl[:], -1.0, 1.0,
                                        op0=ALU.mult, op1=ALU.add)

                for u in range(UNFOLDS):
                    th = wk.tile([P, HC, BC], f32, tag="th", name="th")
                    nc.scalar.activation(th[:], v[:], ACTF.Tanh)
                    if u == 0 and t > 0:
                        nc.sync.dma_start(out=hout[:, t - 1], in_=th[:])
                        if houtD is not None:
                            for hc in range(HC):
                                nc.sync.dma_start(
                                    out=houtD[:, :, t - 1, hc], in_=th[:, hc])
                    psI = pps.tile([P, HC, BC], f32, tag="psI", name="psI")
                    mmT(psI, wrec, th, HC)
                    usb = wk.tile([P, HC, BC], f32, tag="usb", name="usb")
                    nc.vector.tensor_add(usb[:], psI[:], I0t)
                    s_ = wk.tile([P, HC, BC], f32, tag="s_", name="s_")
                    nc.scalar.activation(s_[:], usb[:], ACTF.Sigmoid)
                    d_ = wk.tile([P, HC, BC], f32, tag="d_", name="d_")
                    nc.vector.tensor_sub(d_[:], s_[:], g[:])
                    e_ = wk.tile([P, HC, BC], f32, tag="e_", name="e_")
                    nc.vector.tensor_mul(e_[:], d_[:], kap[:])
                    nc.vector.tensor_add(g[:], g[:], e_[:])
                    z_ = wk.tile([P, HC, BC], f32, tag="z_", name="z_")
                    nc.vector.tensor_mul(z_[:], g[:], gam[:])
                    w_ = wk.tile([P, HC, BC], f32, tag="w_", name="w_")
                    nc.vector.tensor_mul(w_[:], v[:], cL[:])
                    tp = wk.tile([P, HC, BC], f32, tag="tp", name="tp")
                    nc.vector.scalar_tensor_tensor(tp[:], v[:], 1.0, z_[:],
                                                   op0=ALU.subtract, op1=ALU.mult)
                    vs = wk.tile([P, HC, BC], f32, tag="vs", name="vs")
                    nc.vector.tensor_sub(vs[:], w_[:], tp[:])
                    nc.vector.tensor_scalar(v[:], vs[:], 5.0, -5.0,
                                            op0=ALU.min, op1=ALU.max)
            thL = sst.tile([P, HC, BC], f32, tag="thL", name="thL")
            nc.scalar.activation(thL[:], v[:], ACTF.Tanh)
            nc.sync.dma_start(out=hout[:, Tn - 1], in_=thL[:])
            if houtD is not None:
                for hc in range(HC):
                    nc.sync.dma_start(out=houtD[:, :, Tn - 1, hc], in_=thL[:, hc])

    def ln_proj(tc, hsrc, targets, cst, m_dst=None, r_dst=None, mrd=None):
        """Per nb chunk: stage h cols, LN stats, then per target emit
        rstd*(h @ W'.T - m*RS') + bias' via dst callback."""
        ones_col, ones_row, eps_c = cst
        with tc.tile_pool(name="lnst", bufs=2) as lst, \
             tc.tile_pool(name="lnsm", bufs=2) as lsm, \
             tc.tile_pool(name="lnps", bufs=2, space="PSUM") as lps, \
             tc.tile_pool(name="lnbc", bufs=1, space="PSUM") as lbc, \
             tc.tile_pool(name="lnqs", bufs=1, space="PSUM") as lqs:
            for nb in range(NB):
                t0, t1 = nb * TB, (nb + 1) * TB
                hcs = []
                for hc in range(HC):
                    hsb = lst.tile([P, NBW], f32, tag=f"h{hc}", name="hsb")
                    nc.sync.dma_start(
                        out=hsb[:].rearrange("p (t b) -> p t b", t=TB, b=BC),
                        in_=hsrc[:, t0:t1, hc, :])
                    hcs.append(hsb)
                psS = lqs.tile([1, NBW], f32, tag="psS", name="psS")
                for hc in range(HC):
                    nc.tensor.matmul(psS[:], ones_col[:], hcs[hc][:],
                                     start=(hc == 0), stop=(hc == HC - 1))
                psQ = lqs.tile([1, NBW], f32, tag="psQ", name="psQ")
                for hc in range(HC):
                    sq = lst.tile([P, NBW], f32, tag="sq", name="sq")
                    nc.scalar.activation(sq[:], hcs[hc][:], ACTF.Square)
                    nc.tensor.matmul(psQ[:], ones_col[:], sq[:],
                                     start=(hc == 0), stop=(hc == HC - 1))
                if m_dst is None:
                    m_ = lsm.tile([1, NBW], f32, tag="m_", name="m_")[:]
                    r_ = lsm.tile([1, NBW], f32, tag="r_", name="r_")[:]
                else:
                    m_ = m_dst[:, nb * NBW:(nb + 1) * NBW]
                    r_ = r_dst[:, nb * NBW:(nb + 1) * NBW]
                nc.scalar.activation(m_, psS[:], ACTF.Copy, scale=1.0 / H)
                msq = lsm.tile([1, NBW], f32, tag="msq", name="msq")
                nc.scalar.activation(msq[:], psQ[:], ACTF.Copy, scale=1.0 / H)
                mm_ = lsm.tile([1, NBW], f32, tag="mm_", name="mm_")
                nc.vector.tensor_mul(mm_[:], m_, m_)
                var = lsm.tile([1, NBW], f32, tag="var", name="var")
                nc.vector.tensor_sub(var[:], msq[:], mm_[:])
                std = lsm.tile([1, NBW], f32, tag="std", name="std")
                nc.scalar.activation(std[:], var[:], ACTF.Sqrt, bias=eps_c[:])
                nc.vector.reciprocal_approx_fast(out=r_, in_=std[:])
                psM = lbc.tile([P, NBW], f32, tag="psM", name="psM")
                nc.tensor.matmul(psM[:], ones_row[:], m_, start=True, stop=True)
                mB = lst.tile([P, NBW], f32, tag="mB", name="mB")
                nc.scalar.copy(mB[:], psM[:])
                psR = lbc.tile([P, NBW], f32, tag="psR", name="psR")
                nc.tensor.matmul(psR[:], ones_row[:], r_, start=True, stop=True)
                rB = lst.tile([P, NBW], f32, tag="rB", name="rB")
                nc.scalar.copy(rB[:], psR[:])
                for tg in targets:
                    w_sb, nrs, bias = tg["w"], tg["nrs"], tg["bias"]
                    for hc in range(HC):
                        psP = lps.tile([P, NBW], f32, tag="psP", name="psP")
                        for kc in range(HC):
                            nc.tensor.matmul(
                                psP[:],
                                w_sb[:, kc * H + hc * P: kc * H + hc * P + P],
                                hcs[kc][:],
                                start=(kc == 0), stop=(kc == HC - 1))
                        t2 = lst.tile([P, NBW], f32, tag="t2", name="t2")
                        nc.vector.scalar_tensor_tensor(
                            t2[:], mB[:], nrs[:, hc:hc + 1], psP[:],
                            op0=ALU.mult, op1=ALU.add)
                        f_ = lst.tile([P, NBW], f32, tag="f_", name="f_")
                        nc.vector.tensor_mul(f_[:], t2[:], rB[:])
                        tg["dst"](nb, hc, f_, bias, t0, t1)
                if mrd is not None:
                    nc.sync.dma_start(out=mrd[0:1, nb * NBW:(nb + 1) * NBW], in_=m_)
                    nc.sync.dma_start(out=mrd[1:2, nb * NBW:(nb + 1) * NBW], in_=r_)

    def dst_dram(dram, stg_pool):
        def f(nb, hc, f_, bias, t0, t1):
            stt = stg_pool.tile([P, NBW], f32, tag="stg", name="stt")
            nc.scalar.activation(stt[:], f_[:], ACTF.Identity,
                                 bias=bias[:, hc:hc + 1])
            nc.sync.dma_start(
                out=dram[:, t0:t1, hc, :],
                in_=stt[:].rearrange("p (t b) -> p t b", t=TB, b=BC))
        return f

    # ============ CONTEXT 1: P1 bulk input projections + layer-0 scan ========
    with TileContext(nc) as tc:
        with tc.tile_pool(name="c1dram", bufs=1, space="DRAM") as dp1, \
             tc.tile_pool(name="c1state", bufs=1) as sp1:
            I0d = dp1.tile([P, Tn, HC, BC], f32, tag="I0d", name="I0d")
            X0d = dp1.tile([P, Tn, HC, BC], f32, tag="X0d", name="X0d")
            v0 = sp1.tile([P, HC, BC], f32, name="v0")
            g0 = sp1.tile([P, HC, BC], f32, name="g0")
            nc.vector.memset(v0[:], 0.0)
            nc.vector.memset(g0[:], 0.0)

            with tc.tile_pool(name="p1w", bufs=1) as p1w, \
                 tc.tile_pool(name="p1st", bufs=3) as stg, \
                 tc.tile_pool(name="p1ps", bufs=2, space="PSUM") as pp:
                wb = load(p1w, "win0T", "tau0axT", "b_i0", "b_x0")
                xsb = p1w.tile([P, IN // P, Tn * BC], f32, name="xsb")
                nc.sync.dma_start(out=xsb[:], in_=PR["x_T"][:])
                nkx = IN // P
                for (wnm, bnm, dst) in [("win0T", "b_i0", I0d),
                                        ("tau0axT", "b_x0", X0d)]:
                    for hc in range(HC):
                        for nb in range(NB):
                            ps = pp.tile([P, NBW], f32, tag="ps", name="ps")
                            for kc in range(nkx):
                                nc.tensor.matmul(
                                    ps[:],
                                    wb[wnm][:, kc * H + hc * P: kc * H + hc * P + P],
                                    xsb[:, kc, nb * NBW:(nb + 1) * NBW],
                                    start=(kc == 0), stop=(kc == nkx - 1))
                            stt = stg.tile([P, NBW], f32, tag="st", name="stt")
                            nc.scalar.activation(stt[:], ps[:], ACTF.Identity,
                                                 bias=wb[bnm][:, hc:hc + 1])
                            t0, t1 = nb * TB, (nb + 1) * TB
                            nc.sync.dma_start(
                                out=dst[:, t0:t1, hc, :],
                                in_=stt[:].rearrange("p (t b) -> p t b",
                                                     t=TB, b=BC))

            with tc.tile_pool(name="scanw0", bufs=1) as scw:
                sw = load(scw, "wrec0T", "tau0avT", "tau0bT",
                          "gs0bc", "gl0bc", "tbb0bc")
                scan_layer(tc, 0, sw, v0, g0, I0d, X0d, h0B)
            copy_tap(tc, h0B, "h0")

    # ============ CONTEXT 2: P3 LN0-folded projections + layer-1 scan ========
    with TileContext(nc) as tc:
        with tc.tile_pool(name="c2dram", bufs=1, space="DRAM") as dp2, \
             tc.tile_pool(name="c2const", bufs=1) as cp2, \
             tc.tile_pool(name="c2state", bufs=1) as sp2:
            I1d = dp2.tile([P, Tn, HC, BC], f32, tag="I1d", name="I1d")
            X1d = dp2.tile([P, Tn, HC, BC], f32, tag="X1d", name="X1d")
            cst2 = consts(cp2)
            v1 = sp2.tile([P, HC, BC], f32, name="v1")
            g1 = sp2.tile([P, HC, BC], f32, name="g1")
            nc.vector.memset(v1[:], 0.0)
            nc.vector.memset(g1[:], 0.0)

            with tc.tile_pool(name="p3w", bufs=1) as p3w, \
                 tc.tile_pool(name="p3stg", bufs=3) as p3stg:
                w3 = load(p3w, "win1T", "tau1axT", "nrs_i1", "nrs_x1",
                          "b_i1", "b_x1")
                ln_proj(tc, h0B, [
                    {"w": w3["win1T"], "nrs": w3["nrs_i1"], "bias": w3["b_i1"],
                     "dst": dst_dram(I1d, p3stg)},
                    {"w": w3["tau1axT"], "nrs": w3["nrs_x1"], "bias": w3["b_x1"],
                     "dst": dst_dram(X1d, p3stg)},
                ], cst2)
            copy_tap(tc, I1d, "I1")

            with tc.tile_pool(name="scanw1", bufs=1) as scw:
                sw = load(scw, "wrec1T", "tau1avT", "tau1bT",
                          "gs1bc", "gl1bc", "tbb1bc")
                scan_layer(tc, 1, sw, v1, g1, I1d, X1d, h1B, houtD=h1D)
            copy_tap(tc, h1B, "h1")

    # ============ CONTEXT 3: attention (last timestep) + head ================
    with TileContext(nc) as tc:
        with tc.tile_pool(name="c3dram", bufs=1, space="DRAM") as dp3, \
             tc.tile_pool(name="c3const", bufs=1) as cp3, \
             tc.tile_pool(name="atw", bufs=1) as atw:
            mrd = dp3.tile([2, Tn * BC], f32, tag="mrd", name="mrd")
            cst3 = consts(cp3)
            ones_col, ones_row, eps_c = cst3
            ident = cp3.tile([NH * BC, NH * BC], f32, name="ident")
            make_identity(nc, ident[:])
            m1_sb = cp3.tile([1, Tn * BC], f32, name="m1_sb")
            r1_sb = cp3.tile([1, Tn * BC], f32, name="r1_sb")

            wa = load(atw, "wkT", "wvT", "wqT", "woT", "p1T", "p2T",
                      "b_k", "b_q", "b_o", "b_p1", "b_p2", "nrs_k",
                      "rsv_flat", "bv_flat")
            KT = atw.tile([P, HC * Tn * BC], f32, name="KT")

            def dst_K(nb, hc, f_, bias, t0, t1):
                nc.scalar.activation(
                    KT[:, hc * Tn * BC + nb * NBW: hc * Tn * BC + (nb + 1) * NBW],
                    f_[:], ACTF.Identity, bias=bias[:, hc:hc + 1])

            ln_proj(tc, h1B,
                    [{"w": wa["wkT"], "nrs": wa["nrs_k"], "bias": wa["b_k"],
                      "dst": dst_K}],
                    cst3, m_dst=m1_sb, r_dst=r1_sb, mrd=mrd)
            stap("KTc", KT[:, 0:Tn * BC])

            with tc.tile_pool(name="vstats", bufs=1) as vsp:
                m1T = vsp.tile([P, NT, BC], f32, name="m1T")
                r1T = vsp.tile([P, NT, BC], f32, name="r1T")
                nc.sync.dma_start(
                    out=m1T[:],
                    in_=mrd[0:1, :].rearrange("o (tc p b) -> (o p) tc b",
                                              tc=NT, p=P, b=BC))
                nc.sync.dma_start(
                    out=r1T[:],
                    in_=mrd[1:2, :].rearrange("o (tc p b) -> (o p) tc b",
                                              tc=NT, p=P, b=BC))

                # ---- q_last ----
                qT = vsp.tile([P, HC, BC], f32, name="qT")
                with tc.tile_pool(name="qps", bufs=1, space="PSUM") as qps, \
                     tc.tile_pool(name="qwk", bufs=2) as qwk, \
                     tc.tile_pool(name="qc", bufs=1) as qc:
                    hL = qc.tile([P, HC, BC], f32, name="hL")
                    nc.sync.dma_start(out=hL[:], in_=h1B[:, Tn - 1])
                    psb = qps.tile([P, BC], f32, tag="psb", name="psb")
                    nc.tensor.matmul(psb[:], ones_row[:],
                                     m1_sb[:, (Tn - 1) * BC: Tn * BC],
                                     start=True, stop=True)
                    mL = qc.tile([P, BC], f32, name="mL")
                    nc.scalar.copy(mL[:], psb[:])
                    psb2 = qps.tile([P, BC], f32, tag="psb2", name="psb2")
                    nc.tensor.matmul(psb2[:], ones_row[:],
                                     r1_sb[:, (Tn - 1) * BC: Tn * BC],
                                     start=True, stop=True)
                    rL = qc.tile([P, BC], f32, name="rL")
                    nc.scalar.copy(rL[:], psb2[:])
                    xh = qc.tile([P, HC, BC], f32, name="xh")
                    for hc in range(HC):
                        tt = qwk.tile([P, BC], f32, tag="xh1", name="tt")
                        nc.vector.tensor_sub(tt[:], hL[:, hc], mL[:])
                        nc.vector.tensor_mul(xh[:, hc], tt[:], rL[:])
                    psq = qps.tile([P, HC, BC], f32, tag="psq", name="psq")
                    mmT(psq, wa["wqT"], xh, HC)
                    for hc in range(HC):
                        nc.scalar.activation(qT[:, hc], psq[:, hc], ACTF.Identity,
                                             bias=wa["b_q"][:, hc:hc + 1])
                    stap("xh", xh[:])
                stap("q", qT[:])

                # ---- scores + softmax ----
                sc = vsp.tile([NH * BC, Tn], f32, name="sc")
                with tc.tile_pool(name="scops", bufs=2, space="PSUM") as sps, \
                     tc.tile_pool(name="scowk", bufs=3) as swk:
                    for b in range(BC):
                        for h in range(NH):
                            pss = sps.tile([1, Tn], f32, tag="pss", name="pss")
                            nc.tensor.matmul(
                                pss[:],
                                qT[:, h, b:b + 1],
                                KT[:, h * Tn * BC + b: (h + 1) * Tn * BC: BC],
                                start=True, stop=True)
                            srow = swk.tile([1, Tn], f32, tag="srow", name="srow")
                            nc.scalar.copy(srow[:], pss[:])
                            nc.sync.dma_start(
                                out=sc[b * NH + h: b * NH + h + 1, :], in_=srow[:])
                stap("sc", sc[:])
                mx = vsp.tile([NH * BC, 1], f32, name="mx")
                nc.vector.tensor_reduce(mx[:], sc[:], axis=mybir.AxisListType.X,
                                        op=ALU.max)
                nmx = vsp.tile([NH * BC, 1], f32, name="nmx")
                nc.vector.tensor_scalar(nmx[:], mx[:], -1.0, None, op0=ALU.mult)
                ex = vsp.tile([NH * BC, Tn], f32, name="ex")
                sm = vsp.tile([NH * BC, 1], f32, name="sm")
                nc.scalar.activation(ex[:], sc[:], ACTF.Exp, bias=nmx[:],
                                     accum_out=sm[:])
                rsm = vsp.tile([NH * BC, 1], f32, name="rsm")
                nc.vector.reciprocal_approx_fast(out=rsm[:], in_=sm[:])
                en = vsp.tile([NH * BC, Tn], f32, name="en")
                nc.vector.tensor_scalar(en[:], ex[:], rsm[:], None, op0=ALU.mult)
                stap("en", en[:])
                eT = []
                with tc.tile_pool(name="etps", bufs=2, space="PSUM") as eps_p:
                    for kc in range(NT):
                        pse = eps_p.tile([P, NH * BC], f32, tag="pse", name="pse")
                        nc.tensor.transpose(pse[:], en[:, kc * P:(kc + 1) * P],
                                            ident[:])
                        esb = vsp.tile([P, NH * BC], f32, name=f"eT{kc}",
                                       tag=f"eT{kc}")
                        nc.scalar.copy(esb[:], pse[:])
                        eT.append(esb)

                # ---- V projection (per example) + o ----
                psO_sb = vsp.tile([P, HC, BC], f32, name="psO_sb")
                with tc.tile_pool(name="vps", bufs=2, space="PSUM") as vps, \
                     tc.tile_pool(name="vwk", bufs=2) as vwk, \
                     tc.tile_pool(name="ops", bufs=1, space="PSUM") as ops_:
                    psO = ops_.tile([P, HC, BC], f32, tag="psO", name="psO")
                    for b in range(BC):
                        hb = vwk.tile([P, Tn, HC], f32, tag="hb", name="hb")
                        nc.sync.dma_start(out=hb[:], in_=h1D[:, b])
                        Vb = vwk.tile([P, NT * H], f32, tag="Vb", name="Vb")
                        for tcc in range(NT):
                            psV = vps.tile([P, H], f32, tag="psV", name="psV")
                            for kc in range(HC):
                                nc.tensor.matmul(
                                    psV[:],
                                    hb[:, tcc * P:(tcc + 1) * P, kc],
                                    wa["wvT"][:, kc * H:(kc + 1) * H],
                                    start=(kc == 0), stop=(kc == HC - 1))
                            t2 = vwk.tile([P, H], f32, tag="t2v", name="t2")
                            nc.vector.scalar_tensor_tensor(
                                t2[:], wa["rsv_flat"][:], m1T[:, tcc, b:b + 1],
                                psV[:], op0=ALU.mult, op1=ALU.add)
                            f_ = vwk.tile([P, H], f32, tag="f_v", name="f_")
                            nc.vector.tensor_scalar(
                                f_[:], t2[:], r1T[:, tcc, b:b + 1], None,
                                op0=ALU.mult)
                            nc.vector.tensor_add(
                                Vb[:, tcc * H:(tcc + 1) * H], f_[:],
                                wa["bv_flat"][:])
                        if b == 0:
                            stap("Vb0", Vb[:])
                        for h in range(NH):
                            for kc in range(NT):
                                nc.tensor.matmul(
                                    psO[:, h, b:b + 1],
                                    Vb[:, kc * H + h * HD: kc * H + (h + 1) * HD],
                                    eT[kc][:, b * NH + h: b * NH + h + 1],
                                    start=(kc == 0), stop=(kc == NT - 1))
                    nc.scalar.copy(psO_sb[:], psO[:])
                    stap("oT", psO_sb[:])

                # ---- head ----
                with tc.tile_pool(name="hps", bufs=1, space="PSUM") as hps, \
                     tc.tile_pool(name="hc_", bufs=1) as hcp:
                    psAO = hps.tile([P, HC, BC], f32, tag="psAO", name="psAO")
                    mmT(psAO, wa["woT"], psO_sb, HC)
                    ao = hcp.tile([P, HC, BC], f32, name="ao")
                    for hc in range(HC):
                        nc.scalar.activation(ao[:, hc], psAO[:, hc], ACTF.Identity,
                                             bias=wa["b_o"][:, hc:hc + 1])
                    stap("ao", ao[:])
                    psP1 = hps.tile([P, 2, BC], f32, tag="psP1", name="psP1")
                    for hc in range(2):
                        for kc in range(HC):
                            nc.tensor.matmul(
                                psP1[:, hc],
                                wa["p1T"][:, kc * (H // 2) + hc * P:
                                          kc * (H // 2) + hc * P + P],
                                ao[:, kc],
                                start=(kc == 0), stop=(kc == HC - 1))
                    h1_ = hcp.tile([P, 2, BC], f32, name="h1_")
                    for hc in range(2):
                        nc.scalar.activation(h1_[:, hc], psP1[:, hc], ACTF.Relu,
                                             bias=wa["b_p1"][:, hc:hc + 1])
                    psP2 = hps.tile([P, 2, BC], f32, tag="psP2", name="psP2")
                    for hc in range(2):
                        for kc in range(2):
                            nc.tensor.matmul(
                                psP2[:, hc],
                                wa["p2T"][:, kc * OUT + hc * P:
                                          kc * OUT + hc * P + P],
                                h1_[:, kc],
                                start=(kc == 0), stop=(kc == 1))
                    outT = hcp.tile([P, 2, BC], f32, name="outT")
                    for hc in range(2):
                        nc.scalar.activation(outT[:, hc], psP2[:, hc],
                                             ACTF.Identity,
                                             bias=wa["b_p2"][:, hc:hc + 1])
                    for c in range(2):
                        nc.sync.dma_start(
                            out=out_p[:, c * P:(c + 1) * P].rearrange("b p -> p b"),
                            in_=outT[:, c])

    nc.finalize()
    return nc


# ---------------------------------------------------------------- host driver

def _prep_inputs(inputs, Tn=T):
    d = {k: np.asarray(v, np.float32) for k, v in inputs.items()}
    sqh = np.float32(1.0 / np.sqrt(HD))

    wi1, bi1, rs_i1 = _fold3(d["Win1_w"], d["Win1_b"], d["ln0_w"], d["ln0_b"])
    wx1, bx1, rs_x1 = _fold3(d["tau1a_w"][:, :H], d["tau1a_b"], d["ln0_w"], d["ln0_b"])
    ab = d["attn_in_b"]
    wq, bq, _ = _fold3(d["attn_in_w"][0:H] * sqh, ab[0:H] * sqh, d["ln1_w"], d["ln1_b"])
    wk, bk, rs_k = _fold3(d["attn_in_w"][H:2 * H], ab[H:2 * H], d["ln1_w"], d["ln1_b"])
    wv, bv, rs_v = _fold3(d["attn_in_w"][2 * H:3 * H], ab[2 * H:3 * H],
                          d["ln1_w"], d["ln1_b"])

    common = {
        "win0T": _wT(d["Win0_w"]), "tau0axT": _wT(d["tau0a_w"][:, :IN]),
        "tau0avT": _wT(d["tau0a_w"][:, IN:]), "tau0bT": _wT(d["tau0b_w"]),
        "wrec0T": _wT(d["Wrec0_w"]),
        "win1T": _wT(wi1), "tau1axT": _wT(wx1),
        "tau1avT": _wT(d["tau1a_w"][:, H:]), "tau1bT": _wT(d["tau1b_w"]),
        "wrec1T": _wT(d["Wrec1_w"]),
        "wkT": _wT(wk), "wvT": _wT(wv), "wqT": _wT(wq),
        "woT": _wT(d["attn_out_w"]), "p1T": _wT(d["p1_w"]), "p2T": _wT(d["p2_w"]),
        "b_i0": _perH(d["Win0_b"]), "b_x0": _perH(d["tau0a_b"]),
        "b_i1": _perH(bi1), "b_x1": _perH(bx1),
        "b_k": _perH(bk), "b_q": _perH(bq),
        "b_o": _perH(d["attn_out_b"]),
        "b_p1": _perH(d["p1_b"]), "b_p2": _perH(d["p2_b"]),
        "nrs_i1": _perH(-rs_i1), "nrs_x1": _perH(-rs_x1), "nrs_k": _perH(-rs_k),
        "gs0bc": _bcast(DT * d["gsyn0"]), "gl0bc": _bcast(DT * d["gleak0"]),
        "tbb0bc": _bcast(d["tau0b_b"]),
        "gs1bc": _bcast(DT * d["gsyn1"]), "gl1bc": _bcast(DT * d["gleak1"]),
        "tbb1bc": _bcast(d["tau1b_b"]),
        "rsv_flat": np.ascontiguousarray(
            np.broadcast_to((-rs_v)[None, :], (P, H))).astype(np.float32),
        "bv_flat": np.ascontiguousarray(
            np.broadcast_to(bv[None, :], (P, H))).astype(np.float32),
    }
    x = d["inputs"][:, :Tn]
    in_maps = []
    for c in range(NCORES):
        m = dict(common)
        m["x_T"] = _xT(x[c * BC:(c + 1) * BC])
        in_maps.append(m)
    return in_maps


def _run(inputs, trace=False, Tn=T, taps=()):
    from concourse.bass_utils import run_bass_kernel_spmd
    key = (Tn, tuple(taps))
    if key not in _CACHE:
        _CACHE[key] = _build(Tn, taps)
    nc = _CACHE[key]
    in_maps = _prep_inputs(inputs, Tn)
    res = run_bass_kernel_spmd(nc, in_maps, list(range(NCORES)), trace=trace)
    outs = [r["out"] for r in res.results]
    full = np.concatenate(outs, axis=0).astype(np.float32)
    return full, res


def kernel(**inputs):
    out, _ = _run(inputs, trace=False)
    return out


# revision 22
# speedup vs baseline: 3.1233x; 3.1233x over previous
"""Trainium2 Bass kernel for nn_AdaptiveLNN (2x LTC recurrent layers + MHA + head).

Strategy:
  - Pure data parallelism: B=64 sharded over 8 NeuronCores (Bc=8/core), zero
    collectives. Each core runs the full scan for its shard.
  - Transposed layout on chip: activations stored (128 part = h%128,
    free = (h_chunk, b)) -> tiles (128, 4, 8).
  - Input-dependent matmuls (x @ Win.T, x-part of tau_a) precomputed in bulk.
  - LayerNorm never materialized: all consumers are linear, so LN folds into
    the following matmul (host folds ln_w/ln_b into weights; m/rstd folded on
    chip:  LN(h) @ W'.T = rstd*(h @ W'.T - m*rowsum(W')) + bias').
  - Attention: reference uses only the LAST timestep of attention output, so
    only q[T-1] is needed -> O(T) attention.
  - The program is split into 3 sequential TileContexts (P1+scan0 | P3+scan1 |
    attention) so per-semaphore increment counts stay under the HW ceiling;
    contexts hand data across via raw DRAM tensors (ordered by the all-engine
    barrier at each TileContext exit).
"""

import numpy as np

B, T, IN, H, OUT, NH = 64, 512, 256, 512, 256, 4
HD = H // NH
DT = 0.1
UNFOLDS = 6
MIN_TAU, MAX_TAU = 0.1, 10.0
NCORES = 8
BC = B // NCORES          # 8
HC = H // 128             # 4
P = 128
FW = HC * BC              # 32
EPS = 1e-5

_CACHE = {}


# ---------------------------------------------------------------- host packing

def _wT(Wt):
    """(out_f, in_f) -> lhsT sbuf layout (128, nk*out_f):
    [p, kc*out_f + m] = W[m, kc*128 + p]."""
    Wt = np.ascontiguousarray(Wt, np.float32)
    of, inf_ = Wt.shape
    nk = inf_ // P
    a = Wt.T.reshape(nk, P, of)
    return np.ascontiguousarray(a.transpose(1, 0, 2).reshape(P, nk * of))


def _bcast(vec):
    """(H,) -> (128, HC, BC): [p, hc, b] = vec[hc*128+p]."""
    a = np.asarray(vec, np.float32).reshape(HC, P).T
    return np.ascontiguousarray(
        np.repeat(a[:, :, None], BC, axis=2).reshape(P, HC, BC))


def _perH(vec):
    """(F,) -> (128, F//128): [p, c] = vec[c*128+p]."""
    v = np.asarray(vec, np.float32)
    return np.ascontiguousarray(v.reshape(v.size // P, P).T)


def _xT(x):
    """(Bc, Tn, F) -> (128, F//128, Tn*Bc): [p, kc, t*Bc+b] = x[b, t, kc*128+p]."""
    Bc, Tn, F = x.shape
    nk = F // P
    a = x.transpose(2, 1, 0).reshape(nk, P, Tn, Bc)
    return np.ascontiguousarray(
        a.transpose(1, 0, 2, 3).reshape(P, nk, Tn * Bc).astype(np.float32))


def _fold3(Wt, bias, ln_w, ln_b):
    """Fold input-LN affine into weight/bias; return (W', bias', rowsum(W'))."""
    Wt = np.asarray(Wt, np.float32)
    Wp = Wt * np.asarray(ln_w, np.float32)[None, :]
    bp = np.asarray(bias, np.float32) + Wt @ np.asarray(ln_b, np.float32)
    return Wp, bp, Wp.sum(axis=1)


# ---------------------------------------------------------------- builder

def _build(Tn=T, taps=()):
    import concourse.bass as bass
    import concourse.mybir as mybir
    from concourse import bacc
    from concourse.tile import TileContext
    from concourse.masks import make_identity

    f32 = mybir.dt.float32
    ALU = mybir.AluOpType
    ACTF = mybir.ActivationFunctionType

    NB = max(1, (Tn * BC) // 512)      # bulk N-chunks over (t, b)
    NBW = (Tn * BC) // NB              # bulk N width (<= 512)
    TB = NBW // BC                     # timesteps per bulk chunk
    NT = Tn // P                       # t-chunks of 128
    CH = 16                            # scan stream chunk (steps)

    nc = bacc.Bacc("TRN2", target_bir_lowering=False)
    bf16 = mybir.dt.bfloat16

    def par(name, shape):
        dt_ = bf16 if name in BF16_PARAMS else f32
        return nc.declare_dram_parameter(name, list(shape), dt_, isOutput=False)

    BF16_PARAMS = {"wrec0T", "tau0avT", "tau0bT", "wrec1T", "tau1avT", "tau1bT"}
    PARAMS = [
        ("x_T", (P, IN // P, Tn * BC)),
        ("win0T", (P, (IN // P) * H)), ("tau0axT", (P, (IN // P) * H)),
        ("tau0avT", (P, HC * H)), ("tau0bT", (P, HC * H)), ("wrec0T", (P, HC * H)),
        ("win1T", (P, HC * H)), ("tau1axT", (P, HC * H)),
        ("tau1avT", (P, HC * H)), ("tau1bT", (P, HC * H)), ("wrec1T", (P, HC * H)),
        ("wkT", (P, HC * H)), ("wvT", (P, HC * H)), ("wqT", (P, HC * H)),
        ("woT", (P, HC * H)), ("p1T", (P, HC * (H // 2))), ("p2T", (P, 2 * OUT)),
        ("b_i0", (P, HC)), ("b_x0", (P, HC)),
        ("b_i1", (P, HC)), ("b_x1", (P, HC)),
        ("b_k", (P, HC)), ("b_q", (P, HC)),
        ("b_o", (P, HC)), ("b_p1", (P, 2)), ("b_p2", (P, 2)),
        ("nrs_i1", (P, HC)), ("nrs_x1", (P, HC)), ("nrs_k", (P, HC)),
        ("gs0bc", (P, HC, BC)), ("gl0bc", (P, HC, BC)), ("tbb0bc", (P, HC, BC)),
        ("gs1bc", (P, HC, BC)), ("gl1bc", (P, HC, BC)), ("tbb1bc", (P, HC, BC)),
        ("rsv_flat", (P, H)), ("bv_flat", (P, H)),
    ]
    PR = {name: par(name, shape) for name, shape in PARAMS}
    out_p = nc.declare_dram_parameter("out", [BC, OUT], f32, isOutput=True)
    tap_h = {}
    SMALL_TAPS = {
        "q": [P, HC, BC], "sc": [NH * BC, Tn], "en": [NH * BC, Tn],
        "oT": [P, HC, BC], "ao": [P, HC, BC], "Vb0": [P, NT * H],
        "KTc": [P, Tn * BC], "xh": [P, HC, BC],
    }
    stp = {}
    for tname in taps:
        if tname in SMALL_TAPS:
            stp[tname] = nc.declare_dram_parameter(
                "tap_" + tname, SMALL_TAPS[tname], f32, isOutput=True)
        else:
            tap_h[tname] = nc.declare_dram_parameter(
                "tap_" + tname, [P, Tn, HC, BC], f32, isOutput=True)

    # Cross-context intermediates (ordered by TileContext exit barriers).
    h0B = nc.dram_tensor("h0B", [P, Tn, HC, BC], f32)
    h1B = nc.dram_tensor("h1B", [P, Tn, HC, BC], f32)
    h1D = nc.dram_tensor("h1D", [P, BC, Tn, HC], f32)

    # ---------------- shared helpers ----------------
    def load(pool, *names):
        out = {}
        for nm in names:
            t_ = pool.tile(list(PR[nm].shape), PR[nm].dtype, tag=nm, name=nm)
            nc.sync.dma_start(out=t_[:], in_=PR[nm][:])
            out[nm] = t_
        return out

    def mmT(ps, w_sb, rhs, nk, hcs=HC, wof=H):
        for hc in range(hcs):
            for kc in range(nk):
                nc.tensor.matmul(
                    ps[:, hc],
                    w_sb[:, kc * wof + hc * P: kc * wof + hc * P + P],
                    rhs[:, kc],
                    start=(kc == 0), stop=(kc == nk - 1))

    def consts(cp):
        ones_col = cp.tile([P, 1], f32, name="ones_col")
        nc.vector.memset(ones_col[:], 1.0)
        ones_row = cp.tile([1, P], f32, name="ones_row")
        nc.vector.memset(ones_row[:], 1.0)
        eps_c = cp.tile([1, 1], f32, name="eps_c")
        nc.vector.memset(eps_c[:], EPS)
        return ones_col, ones_row, eps_c

    def stap(name, ap):
        if name in stp:
            nc.sync.dma_start(out=stp[name][tuple(
                slice(0, s) for s in stp[name].shape)], in_=ap)

    def copy_tap(tc, src, name):
        if name in tap_h:
            with tc.tile_pool(name="tap" + name, bufs=2) as tpp:
                for t in range(Tn):
                    tt = tpp.tile([P, HC, BC], f32, tag="t", name="tt")
                    nc.sync.dma_start(out=tt[:], in_=src[:, t])
                    nc.sync.dma_start(out=tap_h[name][:, t], in_=tt[:])

    def scan_layer(tc, layer, sw, v, g, Isrc, Xsrc, hout, houtD=None):
        L = str(layer)
        wrec, tauav, taub = sw["wrec" + L + "T"], sw["tau" + L + "avT"], sw["tau" + L + "bT"]
        gsbc, glbc, tbbbc = sw["gs" + L + "bc"], sw["gl" + L + "bc"], sw["tbb" + L + "bc"]
        with tc.tile_pool(name="scps" + L, bufs=2, space="PSUM") as pps, \
             tc.tile_pool(name="scwk" + L, bufs=3) as wk, \
             tc.tile_pool(name="scst" + L, bufs=2) as sst:
            ich = xch = None
            for t in range(Tn):
                if t % CH == 0:
                    ich = sst.tile([P, CH, HC, BC], f32, tag="ich", name="ich")
                    xch = sst.tile([P, CH, HC, BC], f32, tag="xch", name="xch")
                    nc.sync.dma_start(out=ich[:], in_=Isrc[:, t:t + CH])
                    nc.sync.dma_start(out=xch[:], in_=Xsrc[:, t:t + CH])
                I0t = ich[:, t % CH]
                X0t = xch[:, t % CH]

                # tau net (uses v at step start)
                vb = wk.tile([P, HC, BC], bf16, tag="vb", name="vb")
                nc.vector.tensor_copy(vb[:], v[:])
                psA = pps.tile([P, HC, BC], f32, tag="psA", name="psA")
                mmT(psA, tauav, vb, HC)
                u2 = wk.tile([P, HC, BC], f32, tag="u2", name="u2")
                nc.vector.tensor_add(u2[:], psA[:], X0t)
                th2 = wk.tile([P, HC, BC], bf16, tag="th2", name="th2")
                nc.scalar.activation(th2[:], u2[:], ACTF.Tanh)
                psB = pps.tile([P, HC, BC], f32, tag="psB", name="psB")
                mmT(psB, taub, th2, HC)
                u3 = wk.tile([P, HC, BC], f32, tag="u3", name="u3")
                nc.vector.tensor_add(u3[:], psB[:], tbbbc[:])
                sig = wk.tile([P, HC, BC], f32, tag="sig", name="sig")
                nc.scalar.activation(sig[:], u3[:], ACTF.Sigmoid)
                tau = wk.tile([P, HC, BC], f32, tag="tau", name="tau")
                nc.vector.tensor_scalar(tau[:], sig[:], MAX_TAU - MIN_TAU,
                                        MIN_TAU, op0=ALU.mult, op1=ALU.add)
                rtau = wk.tile([P, HC, BC], f32, tag="rtau", name="rtau")
                nc.vector.reciprocal_approx_fast(out=rtau[:], in_=tau[:])
                kap = wk.tile([P, HC, BC], f32, tag="kap", name="kap")
                nc.vector.tensor_scalar(kap[:], rtau[:], DT / 0.5, None,
                                        op0=ALU.mult)
                gam = wk.tile([P, HC, BC], f32, tag="gam", name="gam")
                nc.vector.tensor_mul(gam[:], rtau[:], gsbc[:])  # gs pre-scaled DT
                tl = wk.tile([P, HC, BC], f32, tag="tl", name="tl")
                nc.vector.tensor_mul(tl[:], rtau[:], glbc[:])   # gl pre-scaled DT
                cL = wk.tile([P, HC, BC], f32, tag="cL", name="cL")
                nc.vector.tensor_scalar(cL[:], tl[:], -1.0, 1.0,
                                        op0=ALU.mult, op1=ALU.add)

                for u in range(UNFOLDS):
                    th = wk.tile([P, HC, BC], bf16, tag="th", name="th")
                    nc.scalar.activation(th[:], v[:], ACTF.Tanh)
                    if u == 0 and t > 0:
                        th32 = wk.tile([P, HC, BC], f32, tag="th32", name="th32")
                        nc.vector.tensor_copy(th32[:], th[:])
                        nc.sync.dma_start(out=hout[:, t - 1], in_=th32[:])
                        if houtD is not None:
                            for hc in range(HC):
                                nc.sync.dma_start(
                                    out=houtD[:, :, t - 1, hc], in_=th32[:, hc])
                    psI = pps.tile([P, HC, BC], f32, tag="psI", name="psI")
                    mmT(psI, wrec, th, HC)
                    usb = wk.tile([P, HC, BC], f32, tag="usb", name="usb")
                    nc.vector.tensor_add(usb[:], psI[:], I0t)
                    s_ = wk.tile([P, HC, BC], f32, tag="s_", name="s_")
                    nc.scalar.activation(s_[:], usb[:], ACTF.Sigmoid)
                    d_ = wk.tile([P, HC, BC], f32, tag="d_", name="d_")
                    nc.vector.tensor_sub(d_[:], s_[:], g[:])
                    e_ = wk.tile([P, HC, BC], f32, tag="e_", name="e_")
                    nc.vector.tensor_mul(e_[:], d_[:], kap[:])
                    nc.vector.tensor_add(g[:], g[:], e_[:])
                    z_ = wk.tile([P, HC, BC], f32, tag="z_", name="z_")
                    nc.vector.tensor_mul(z_[:], g[:], gam[:])
                    w_ = wk.tile([P, HC, BC], f32, tag="w_", name="w_")
                    nc.vector.tensor_mul(w_[:], v[:], cL[:])
                    tp = wk.tile([P, HC, BC], f32, tag="tp", name="tp")
                    nc.vector.scalar_tensor_tensor(tp[:], v[:], 1.0, z_[:],
                                                   op0=ALU.subtract, op1=ALU.mult)
                    vs = wk.tile([P, HC, BC], f32, tag="vs", name="vs")
                    nc.vector.tensor_sub(vs[:], w_[:], tp[:])
                    nc.vector.tensor_scalar(v[:], vs[:], 5.0, -5.0,
                                            op0=ALU.min, op1=ALU.max)
            thL = sst.tile([P, HC, BC], f32, tag="thL", name="thL")
            nc.scalar.activation(thL[:], v[:], ACTF.Tanh)
            nc.sync.dma_start(out=hout[:, Tn - 1], in_=thL[:])
            if houtD is not None:
                for hc in range(HC):
                    nc.sync.dma_start(out=houtD[:, :, Tn - 1, hc], in_=thL[:, hc])

    def ln_proj(tc, hsrc, targets, cst, m_dst=None, r_dst=None, mrd=None):
        """Per nb chunk: stage h cols, LN stats, then per target emit
        rstd*(h @ W'.T - m*RS') + bias' via dst callback."""
        ones_col, ones_row, eps_c = cst
        with tc.tile_pool(name="lnst", bufs=2) as lst, \
             tc.tile_pool(name="lnsm", bufs=2) as lsm, \
             tc.tile_pool(name="lnps", bufs=2, space="PSUM") as lps, \
             tc.tile_pool(name="lnbc", bufs=1, space="PSUM") as lbc, \
             tc.tile_pool(name="lnqs", bufs=1, space="PSUM") as lqs:
            for nb in range(NB):
                t0, t1 = nb * TB, (nb + 1) * TB
                hcs = []
                for hc in range(HC):
                    hsb = lst.tile([P, NBW], f32, tag=f"h{hc}", name="hsb")
                    nc.sync.dma_start(
                        out=hsb[:].rearrange("p (t b) -> p t b", t=TB, b=BC),
                        in_=hsrc[:, t0:t1, hc, :])
                    hcs.append(hsb)
                psS = lqs.tile([1, NBW], f32, tag="psS", name="psS")
                for hc in range(HC):
                    nc.tensor.matmul(psS[:], ones_col[:], hcs[hc][:],
                                     start=(hc == 0), stop=(hc == HC - 1))
                psQ = lqs.tile([1, NBW], f32, tag="psQ", name="psQ")
                for hc in range(HC):
                    sq = lst.tile([P, NBW], f32, tag="sq", name="sq")
                    nc.scalar.activation(sq[:], hcs[hc][:], ACTF.Square)
                    nc.tensor.matmul(psQ[:], ones_col[:], sq[:],
                                     start=(hc == 0), stop=(hc == HC - 1))
                if m_dst is None:
                    m_ = lsm.tile([1, NBW], f32, tag="m_", name="m_")[:]
                    r_ = lsm.tile([1, NBW], f32, tag="r_", name="r_")[:]
                else:
                    m_ = m_dst[:, nb * NBW:(nb + 1) * NBW]
                    r_ = r_dst[:, nb * NBW:(nb + 1) * NBW]
                nc.scalar.activation(m_, psS[:], ACTF.Copy, scale=1.0 / H)
                msq = lsm.tile([1, NBW], f32, tag="msq", name="msq")
                nc.scalar.activation(msq[:], psQ[:], ACTF.Copy, scale=1.0 / H)
                mm_ = lsm.tile([1, NBW], f32, tag="mm_", name="mm_")
                nc.vector.tensor_mul(mm_[:], m_, m_)
                var = lsm.tile([1, NBW], f32, tag="var", name="var")
                nc.vector.tensor_sub(var[:], msq[:], mm_[:])
                std = lsm.tile([1, NBW], f32, tag="std", name="std")
                nc.scalar.activation(std[:], var[:], ACTF.Sqrt, bias=eps_c[:])
                nc.vector.reciprocal_approx_fast(out=r_, in_=std[:])
                psM = lbc.tile([P, NBW], f32, tag="psM", name="psM")
                nc.tensor.matmul(psM[:], ones_row[:], m_, start=True, stop=True)
                mB = lst.tile([P, NBW], f32, tag="mB", name="mB")
                nc.scalar.copy(mB[:], psM[:])
                psR = lbc.tile([P, NBW], f32, tag="psR", name="psR")
                nc.tensor.matmul(psR[:], ones_row[:], r_, start=True, stop=True)
                rB = lst.tile([P, NBW], f32, tag="rB", name="rB")
                nc.scalar.copy(rB[:], psR[:])
                for tg in targets:
                    w_sb, nrs, bias = tg["w"], tg["nrs"], tg["bias"]
                    for hc in range(HC):
                        psP = lps.tile([P, NBW], f32, tag="psP", name="psP")
                        for kc in range(HC):
                            nc.tensor.matmul(
                                psP[:],
                                w_sb[:, kc * H + hc * P: kc * H + hc * P + P],
                                hcs[kc][:],
                                start=(kc == 0), stop=(kc == HC - 1))
                        t2 = lst.tile([P, NBW], f32, tag="t2", name="t2")
                        nc.vector.scalar_tensor_tensor(
                            t2[:], mB[:], nrs[:, hc:hc + 1], psP[:],
                            op0=ALU.mult, op1=ALU.add)
                        f_ = lst.tile([P, NBW], f32, tag="f_", name="f_")
                        nc.vector.tensor_mul(f_[:], t2[:], rB[:])
                        tg["dst"](nb, hc, f_, bias, t0, t1)
                if mrd is not None:
                    nc.sync.dma_start(out=mrd[0:1, nb * NBW:(nb + 1) * NBW], in_=m_)
                    nc.sync.dma_start(out=mrd[1:2, nb * NBW:(nb + 1) * NBW], in_=r_)

    def dst_dram(dram, stg_pool):
        def f(nb, hc, f_, bias, t0, t1):
            stt = stg_pool.tile([P, NBW], f32, tag="stg", name="stt")
            nc.scalar.activation(stt[:], f_[:], ACTF.Identity,
                                 bias=bias[:, hc:hc + 1])
            nc.sync.dma_start(
                out=dram[:, t0:t1, hc, :],
                in_=stt[:].rearrange("p (t b) -> p t b", t=TB, b=BC))
        return f

    # ============ CONTEXT 1: P1 bulk input projections + layer-0 scan ========
    with TileContext(nc) as tc:
        with tc.tile_pool(name="c1dram", bufs=1, space="DRAM") as dp1, \
             tc.tile_pool(name="c1state", bufs=1) as sp1:
            I0d = dp1.tile([P, Tn, HC, BC], f32, tag="I0d", name="I0d")
            X0d = dp1.tile([P, Tn, HC, BC], f32, tag="X0d", name="X0d")
            v0 = sp1.tile([P, HC, BC], f32, name="v0")
            g0 = sp1.tile([P, HC, BC], f32, name="g0")
            nc.vector.memset(v0[:], 0.0)
            nc.vector.memset(g0[:], 0.0)

            with tc.tile_pool(name="p1w", bufs=1) as p1w, \
                 tc.tile_pool(name="p1st", bufs=3) as stg, \
                 tc.tile_pool(name="p1ps", bufs=2, space="PSUM") as pp:
                wb = load(p1w, "win0T", "tau0axT", "b_i0", "b_x0")
                xsb = p1w.tile([P, IN // P, Tn * BC], f32, name="xsb")
                nc.sync.dma_start(out=xsb[:], in_=PR["x_T"][:])
                nkx = IN // P
                for (wnm, bnm, dst) in [("win0T", "b_i0", I0d),
                                        ("tau0axT", "b_x0", X0d)]:
                    for hc in range(HC):
                        for nb in range(NB):
                            ps = pp.tile([P, NBW], f32, tag="ps", name="ps")
                            for kc in range(nkx):
                                nc.tensor.matmul(
                                    ps[:],
                                    wb[wnm][:, kc * H + hc * P: kc * H + hc * P + P],
                                    xsb[:, kc, nb * NBW:(nb + 1) * NBW],
                                    start=(kc == 0), stop=(kc == nkx - 1))
                            stt = stg.tile([P, NBW], f32, tag="st", name="stt")
                            nc.scalar.activation(stt[:], ps[:], ACTF.Identity,
                                                 bias=wb[bnm][:, hc:hc + 1])
                            t0, t1 = nb * TB, (nb + 1) * TB
                            nc.sync.dma_start(
                                out=dst[:, t0:t1, hc, :],
                                in_=stt[:].rearrange("p (t b) -> p t b",
                                                     t=TB, b=BC))

            with tc.tile_pool(name="scanw0", bufs=1) as scw:
                sw = load(scw, "wrec0T", "tau0avT", "tau0bT",
                          "gs0bc", "gl0bc", "tbb0bc")
                scan_layer(tc, 0, sw, v0, g0, I0d, X0d, h0B)
            copy_tap(tc, h0B, "h0")

    # ============ CONTEXT 2: P3 LN0-folded projections + layer-1 scan ========
    with TileContext(nc) as tc:
        with tc.tile_pool(name="c2dram", bufs=1, space="DRAM") as dp2, \
             tc.tile_pool(name="c2const", bufs=1) as cp2, \
             tc.tile_pool(name="c2state", bufs=1) as sp2:
            I1d = dp2.tile([P, Tn, HC, BC], f32, tag="I1d", name="I1d")
            X1d = dp2.tile([P, Tn, HC, BC], f32, tag="X1d", name="X1d")
            cst2 = consts(cp2)
            v1 = sp2.tile([P, HC, BC], f32, name="v1")
            g1 = sp2.tile([P, HC, BC], f32, name="g1")
            nc.vector.memset(v1[:], 0.0)
            nc.vector.memset(g1[:], 0.0)

            with tc.tile_pool(name="p3w", bufs=1) as p3w, \
                 tc.tile_pool(name="p3stg", bufs=3) as p3stg:
                w3 = load(p3w, "win1T", "tau1axT", "nrs_i1", "nrs_x1",
                          "b_i1", "b_x1")
                ln_proj(tc, h0B, [
                    {"w": w3["win1T"], "nrs": w3["nrs_i1"], "bias": w3["b_i1"],
                     "dst": dst_dram(I1d, p3stg)},
                    {"w": w3["tau1axT"], "nrs": w3["nrs_x1"], "bias": w3["b_x1"],
                     "dst": dst_dram(X1d, p3stg)},
                ], cst2)
            copy_tap(tc, I1d, "I1")

            with tc.tile_pool(name="scanw1", bufs=1) as scw:
                sw = load(scw, "wrec1T", "tau1avT", "tau1bT",
                          "gs1bc", "gl1bc", "tbb1bc")
                scan_layer(tc, 1, sw, v1, g1, I1d, X1d, h1B, houtD=h1D)
            copy_tap(tc, h1B, "h1")

    # ============ CONTEXT 3: attention (last timestep) + head ================
    with TileContext(nc) as tc:
        with tc.tile_pool(name="c3dram", bufs=1, space="DRAM") as dp3, \
             tc.tile_pool(name="c3const", bufs=1) as cp3, \
             tc.tile_pool(name="atw", bufs=1) as atw:
            mrd = dp3.tile([2, Tn * BC], f32, tag="mrd", name="mrd")
            cst3 = consts(cp3)
            ones_col, ones_row, eps_c = cst3
            ident = cp3.tile([NH * BC, NH * BC], f32, name="ident")
            make_identity(nc, ident[:])
            m1_sb = cp3.tile([1, Tn * BC], f32, name="m1_sb")
            r1_sb = cp3.tile([1, Tn * BC], f32, name="r1_sb")

            wa = load(atw, "wkT", "wvT", "wqT", "woT", "p1T", "p2T",
                      "b_k", "b_q", "b_o", "b_p1", "b_p2", "nrs_k",
                      "rsv_flat", "bv_flat")
            KT = atw.tile([P, HC * Tn * BC], f32, name="KT")

            def dst_K(nb, hc, f_, bias, t0, t1):
                nc.scalar.activation(
                    KT[:, hc * Tn * BC + nb * NBW: hc * Tn * BC + (nb + 1) * NBW],
                    f_[:], ACTF.Identity, bias=bias[:, hc:hc + 1])

            ln_proj(tc, h1B,
                    [{"w": wa["wkT"], "nrs": wa["nrs_k"], "bias": wa["b_k"],
                      "dst": dst_K}],
                    cst3, m_dst=m1_sb, r_dst=r1_sb, mrd=mrd)
            stap("KTc", KT[:, 0:Tn * BC])

            with tc.tile_pool(name="vstats", bufs=1) as vsp:
                m1T = vsp.tile([P, NT, BC], f32, name="m1T")
                r1T = vsp.tile([P, NT, BC], f32, name="r1T")
                nc.sync.dma_start(
                    out=m1T[:],
                    in_=mrd[0:1, :].rearrange("o (tc p b) -> (o p) tc b",
                                              tc=NT, p=P, b=BC))
                nc.sync.dma_start(
                    out=r1T[:],
                    in_=mrd[1:2, :].rearrange("o (tc p b) -> (o p) tc b",
                                              tc=NT, p=P, b=BC))

                # ---- q_last ----
                qT = vsp.tile([P, HC, BC], f32, name="qT")
                with tc.tile_pool(name="qps", bufs=1, space="PSUM") as qps, \
                     tc.tile_pool(name="qwk", bufs=2) as qwk, \
                     tc.tile_pool(name="qc", bufs=1) as qc:
                    hL = qc.tile([P, HC, BC], f32, name="hL")
                    nc.sync.dma_start(out=hL[:], in_=h1B[:, Tn - 1])
                    psb = qps.tile([P, BC], f32, tag="psb", name="psb")
                    nc.tensor.matmul(psb[:], ones_row[:],
                                     m1_sb[:, (Tn - 1) * BC: Tn * BC],
                                     start=True, stop=True)
                    mL = qc.tile([P, BC], f32, name="mL")
                    nc.scalar.copy(mL[:], psb[:])
                    psb2 = qps.tile([P, BC], f32, tag="psb2", name="psb2")
                    nc.tensor.matmul(psb2[:], ones_row[:],
                                     r1_sb[:, (Tn - 1) * BC: Tn * BC],
                                     start=True, stop=True)
                    rL = qc.tile([P, BC], f32, name="rL")
                    nc.scalar.copy(rL[:], psb2[:])
                    xh = qc.tile([P, HC, BC], f32, name="xh")
                    for hc in range(HC):
                        tt = qwk.tile([P, BC], f32, tag="xh1", name="tt")
                        nc.vector.tensor_sub(tt[:], hL[:, hc], mL[:])
                        nc.vector.tensor_mul(xh[:, hc], tt[:], rL[:])
                    psq = qps.tile([P, HC, BC], f32, tag="psq", name="psq")
                    mmT(psq, wa["wqT"], xh, HC)
                    for hc in range(HC):
                        nc.scalar.activation(qT[:, hc], psq[:, hc], ACTF.Identity,
                                             bias=wa["b_q"][:, hc:hc + 1])
                    stap("xh", xh[:])
                stap("q", qT[:])

                # ---- scores + softmax ----
                sc = vsp.tile([NH * BC, Tn], f32, name="sc")
                with tc.tile_pool(name="scops", bufs=2, space="PSUM") as sps, \
                     tc.tile_pool(name="scowk", bufs=3) as swk:
                    for b in range(BC):
                        for h in range(NH):
                            pss = sps.tile([1, Tn], f32, tag="pss", name="pss")
                            nc.tensor.matmul(
                                pss[:],
                                qT[:, h, b:b + 1],
                                KT[:, h * Tn * BC + b: (h + 1) * Tn * BC: BC],
                                start=True, stop=True)
                            srow = swk.tile([1, Tn], f32, tag="srow", name="srow")
                            nc.scalar.copy(srow[:], pss[:])
                            nc.sync.dma_start(
                                out=sc[b * NH + h: b * NH + h + 1, :], in_=srow[:])
                stap("sc", sc[:])
                mx = vsp.tile([NH * BC, 1], f32, name="mx")
                nc.vector.tensor_reduce(mx[:], sc[:], axis=mybir.AxisListType.X,
                                        op=ALU.max)
                nmx = vsp.tile([NH * BC, 1], f32, name="nmx")
                nc.vector.tensor_scalar(nmx[:], mx[:], -1.0, None, op0=ALU.mult)
                ex = vsp.tile([NH * BC, Tn], f32, name="ex")
                sm = vsp.tile([NH * BC, 1], f32, name="sm")
                nc.scalar.activation(ex[:], sc[:], ACTF.Exp, bias=nmx[:],
                                     accum_out=sm[:])
                rsm = vsp.tile([NH * BC, 1], f32, name="rsm")
                nc.vector.reciprocal_approx_fast(out=rsm[:], in_=sm[:])
                en = vsp.tile([NH * BC, Tn], f32, name="en")
                nc.vector.tensor_scalar(en[:], ex[:], rsm[:], None, op0=ALU.mult)
                stap("en", en[:])
                eT = []
                with tc.tile_pool(name="etps", bufs=2, space="PSUM") as eps_p:
                    for kc in range(NT):
                        pse = eps_p.tile([P, NH * BC], f32, tag="pse", name="pse")
                        nc.tensor.transpose(pse[:], en[:, kc * P:(kc + 1) * P],
                                            ident[:])
                        esb = vsp.tile([P, NH * BC], f32, name=f"eT{kc}",
                                       tag=f"eT{kc}")
                        nc.scalar.copy(esb[:], pse[:])
                        eT.append(esb)

                # ---- V projection (per example) + o ----
                psO_sb = vsp.tile([P, HC, BC], f32, name="psO_sb")
                with tc.tile_pool(name="vps", bufs=2, space="PSUM") as vps, \
                     tc.tile_pool(name="vwk", bufs=2) as vwk, \
                     tc.tile_pool(name="ops", bufs=1, space="PSUM") as ops_:
                    psO = ops_.tile([P, HC, BC], f32, tag="psO", name="psO")
                    for b in range(BC):
                        hb = vwk.tile([P, Tn, HC], f32, tag="hb", name="hb")
                        nc.sync.dma_start(out=hb[:], in_=h1D[:, b])
                        Vb = vwk.tile([P, NT * H], f32, tag="Vb", name="Vb")
                        for tcc in range(NT):
                            psV = vps.tile([P, H], f32, tag="psV", name="psV")
                            for kc in range(HC):
                                nc.tensor.matmul(
                                    psV[:],
                                    hb[:, tcc * P:(tcc + 1) * P, kc],
                                    wa["wvT"][:, kc * H:(kc + 1) * H],
                                    start=(kc == 0), stop=(kc == HC - 1))
                            t2 = vwk.tile([P, H], f32, tag="t2v", name="t2")
                            nc.vector.scalar_tensor_tensor(
                                t2[:], wa["rsv_flat"][:], m1T[:, tcc, b:b + 1],
                                psV[:], op0=ALU.mult, op1=ALU.add)
                            f_ = vwk.tile([P, H], f32, tag="f_v", name="f_")
                            nc.vector.tensor_scalar(
                                f_[:], t2[:], r1T[:, tcc, b:b + 1], None,
                                op0=ALU.mult)
                            nc.vector.tensor_add(
                                Vb[:, tcc * H:(tcc + 1) * H], f_[:],
                                wa["bv_flat"][:])
                        if b == 0:
                            stap("Vb0", Vb[:])
                        for h in range(NH):
                            for kc in range(NT):
                                nc.tensor.matmul(
                                    psO[:, h, b:b + 1],
                                    Vb[:, kc * H + h * HD: kc * H + (h + 1) * HD],
                                    eT[kc][:, b * NH + h: b * NH + h + 1],
                                    start=(kc == 0), stop=(kc == NT - 1))
                    nc.scalar.copy(psO_sb[:], psO[:])
                    stap("oT", psO_sb[:])

                # ---- head ----
                with tc.tile_pool(name="hps", bufs=1, space="PSUM") as hps, \
                     tc.tile_pool(name="hc_", bufs=1) as hcp:
                    psAO = hps.tile([P, HC, BC], f32, tag="psAO", name="psAO")
                    mmT(psAO, wa["woT"], psO_sb, HC)
                    ao = hcp.tile([P, HC, BC], f32, name="ao")
                    for hc in range(HC):
                        nc.scalar.activation(ao[:, hc], psAO[:, hc], ACTF.Identity,
                                             bias=wa["b_o"][:, hc:hc + 1])
                    stap("ao", ao[:])
                    psP1 = hps.tile([P, 2, BC], f32, tag="psP1", name="psP1")
                    for hc in range(2):
                        for kc in range(HC):
                            nc.tensor.matmul(
                                psP1[:, hc],
                                wa["p1T"][:, kc * (H // 2) + hc * P:
                                          kc * (H // 2) + hc * P + P],
                                ao[:, kc],
                                start=(kc == 0), stop=(kc == HC - 1))
                    h1_ = hcp.tile([P, 2, BC], f32, name="h1_")
                    for hc in range(2):
                        nc.scalar.activation(h1_[:, hc], psP1[:, hc], ACTF.Relu,
                                             bias=wa["b_p1"][:, hc:hc + 1])
                    psP2 = hps.tile([P, 2, BC], f32, tag="psP2", name="psP2")
                    for hc in range(2):
                        for kc in range(2):
                            nc.tensor.matmul(
                                psP2[:, hc],
                                wa["p2T"][:, kc * OUT + hc * P:
                                          kc * OUT + hc * P + P],
                                h1_[:, kc],
                                start=(kc == 0), stop=(kc == 1))
                    outT = hcp.tile([P, 2, BC], f32, name="outT")
                    for hc in range(2):
                        nc.scalar.activation(outT[:, hc], psP2[:, hc],
                                             ACTF.Identity,
                                             bias=wa["b_p2"][:, hc:hc + 1])
                    for c in range(2):
                        nc.sync.dma_start(
                            out=out_p[:, c * P:(c + 1) * P].rearrange("b p -> p b"),
                            in_=outT[:, c])

    nc.finalize()
    return nc


# ---------------------------------------------------------------- host driver

def _prep_inputs(inputs, Tn=T):
    d = {k: np.asarray(v, np.float32) for k, v in inputs.items()}
    sqh = np.float32(1.0 / np.sqrt(HD))

    wi1, bi1, rs_i1 = _fold3(d["Win1_w"], d["Win1_b"], d["ln0_w"], d["ln0_b"])
    wx1, bx1, rs_x1 = _fold3(d["tau1a_w"][:, :H], d["tau1a_b"], d["ln0_w"], d["ln0_b"])
    ab = d["attn_in_b"]
    wq, bq, _ = _fold3(d["attn_in_w"][0:H] * sqh, ab[0:H] * sqh, d["ln1_w"], d["ln1_b"])
    wk, bk, rs_k = _fold3(d["attn_in_w"][H:2 * H], ab[H:2 * H], d["ln1_w"], d["ln1_b"])
    wv, bv, rs_v = _fold3(d["attn_in_w"][2 * H:3 * H], ab[2 * H:3 * H],
                          d["ln1_w"], d["ln1_b"])

    common = {
        "win0T": _wT(d["Win0_w"]), "tau0axT": _wT(d["tau0a_w"][:, :IN]),
        "tau0avT": _wT(d["tau0a_w"][:, IN:]), "tau0bT": _wT(d["tau0b_w"]),
        "wrec0T": _wT(d["Wrec0_w"]),
        "win1T": _wT(wi1), "tau1axT": _wT(wx1),
        "tau1avT": _wT(d["tau1a_w"][:, H:]), "tau1bT": _wT(d["tau1b_w"]),
        "wrec1T": _wT(d["Wrec1_w"]),
        "wkT": _wT(wk), "wvT": _wT(wv), "wqT": _wT(wq),
        "woT": _wT(d["attn_out_w"]), "p1T": _wT(d["p1_w"]), "p2T": _wT(d["p2_w"]),
        "b_i0": _perH(d["Win0_b"]), "b_x0": _perH(d["tau0a_b"]),
        "b_i1": _perH(bi1), "b_x1": _perH(bx1),
        "b_k": _perH(bk), "b_q": _perH(bq),
        "b_o": _perH(d["attn_out_b"]),
        "b_p1": _perH(d["p1_b"]), "b_p2": _perH(d["p2_b"]),
        "nrs_i1": _perH(-rs_i1), "nrs_x1": _perH(-rs_x1), "nrs_k": _perH(-rs_k),
        "gs0bc": _bcast(DT * d["gsyn0"]), "gl0bc": _bcast(DT * d["gleak0"]),
        "tbb0bc": _bcast(d["tau0b_b"]),
        "gs1bc": _bcast(DT * d["gsyn1"]), "gl1bc": _bcast(DT * d["gleak1"]),
        "tbb1bc": _bcast(d["tau1b_b"]),
        "rsv_flat": np.ascontiguousarray(
            np.broadcast_to((-rs_v)[None, :], (P, H))).astype(np.float32),
        "bv_flat": np.ascontiguousarray(
            np.broadcast_to(bv[None, :], (P, H))).astype(np.float32),
    }
    import ml_dtypes
    for nm in ("wrec0T", "tau0avT", "tau0bT", "wrec1T", "tau1avT", "tau1bT"):
        common[nm] = common[nm].astype(ml_dtypes.bfloat16)
    x = d["inputs"][:, :Tn]
    in_maps = []
    for c in range(NCORES):
        m = dict(common)
        m["x_T"] = _xT(x[c * BC:(c + 1) * BC])
        in_maps.append(m)
    return in_maps


def _run(inputs, trace=False, Tn=T, taps=()):
    from concourse.bass_utils import run_bass_kernel_spmd
    key = (Tn, tuple(taps))
    if key not in _CACHE:
        _CACHE[key] = _build(Tn, taps)
    nc = _CACHE[key]
    in_maps = _prep_inputs(inputs, Tn)
    res = run_bass_kernel_spmd(nc, in_maps, list(range(NCORES)), trace=trace)
    outs = [r["out"] for r in res.results]
    full = np.concatenate(outs, axis=0).astype(np.float32)
    return full, res


def kernel(**inputs):
    out, _ = _run(inputs, trace=False)
    return out


# revision 23
# speedup vs baseline: 3.7801x; 1.2103x over previous
"""Trainium2 Bass kernel for nn_AdaptiveLNN (2x LTC recurrent layers + MHA + head).

Strategy:
  - Pure data parallelism: B=64 sharded over 8 NeuronCores (Bc=8/core), zero
    collectives. Each core runs the full scan for its shard.
  - Transposed layout on chip: activations stored (128 part = h%128,
    free = (h_chunk, b)) -> tiles (128, 4, 8).
  - Input-dependent matmuls (x @ Win.T, x-part of tau_a) precomputed in bulk.
  - LayerNorm never materialized: all consumers are linear, so LN folds into
    the following matmul (host folds ln_w/ln_b into weights; m/rstd folded on
    chip:  LN(h) @ W'.T = rstd*(h @ W'.T - m*rowsum(W')) + bias').
  - Attention: reference uses only the LAST timestep of attention output, so
    only q[T-1] is needed -> O(T) attention.
  - The program is split into 3 sequential TileContexts (P1+scan0 | P3+scan1 |
    attention) so per-semaphore increment counts stay under the HW ceiling;
    contexts hand data across via raw DRAM tensors (ordered by the all-engine
    barrier at each TileContext exit).
"""

import numpy as np

B, T, IN, H, OUT, NH = 64, 512, 256, 512, 256, 4
HD = H // NH
DT = 0.1
UNFOLDS = 6
MIN_TAU, MAX_TAU = 0.1, 10.0
NCORES = 8
BC = B // NCORES          # 8
HC = H // 128             # 4
P = 128
FW = HC * BC              # 32
EPS = 1e-5

_CACHE = {}


# ---------------------------------------------------------------- host packing

def _wT(Wt):
    """(out_f, in_f) -> lhsT sbuf layout (128, nk*out_f):
    [p, kc*out_f + m] = W[m, kc*128 + p]."""
    Wt = np.ascontiguousarray(Wt, np.float32)
    of, inf_ = Wt.shape
    nk = inf_ // P
    a = Wt.T.reshape(nk, P, of)
    return np.ascontiguousarray(a.transpose(1, 0, 2).reshape(P, nk * of))


def _bcast(vec):
    """(H,) -> (128, HC, BC): [p, hc, b] = vec[hc*128+p]."""
    a = np.asarray(vec, np.float32).reshape(HC, P).T
    return np.ascontiguousarray(
        np.repeat(a[:, :, None], BC, axis=2).reshape(P, HC, BC))


def _perH(vec):
    """(F,) -> (128, F//128): [p, c] = vec[c*128+p]."""
    v = np.asarray(vec, np.float32)
    return np.ascontiguousarray(v.reshape(v.size // P, P).T)


def _xT(x):
    """(Bc, Tn, F) -> (128, F//128, Tn*Bc): [p, kc, t*Bc+b] = x[b, t, kc*128+p]."""
    Bc, Tn, F = x.shape
    nk = F // P
    a = x.transpose(2, 1, 0).reshape(nk, P, Tn, Bc)
    return np.ascontiguousarray(
        a.transpose(1, 0, 2, 3).reshape(P, nk, Tn * Bc).astype(np.float32))


def _fold3(Wt, bias, ln_w, ln_b):
    """Fold input-LN affine into weight/bias; return (W', bias', rowsum(W'))."""
    Wt = np.asarray(Wt, np.float32)
    Wp = Wt * np.asarray(ln_w, np.float32)[None, :]
    bp = np.asarray(bias, np.float32) + Wt @ np.asarray(ln_b, np.float32)
    return Wp, bp, Wp.sum(axis=1)


_SUBCLIP = None


def _register_custom_dve():
    """Register a fused out = clip(in0 - in1, s0, s1) DVE op (one instruction
    replacing subtract + min/max)."""
    global _SUBCLIP
    if _SUBCLIP is not None:
        return _SUBCLIP
    from concourse.dve_spec import Spec, lower, minn, maxx, Src0, Src1, C0, C1
    from concourse.dve_uop import DveOpSpec
    from concourse import dve_ops
    for o in dve_ops.OPS:
        if o.name == "SUB_CLIP_ANT":
            _SUBCLIP = o
            return o
    spec = Spec(
        body=minn(maxx(Src0 - Src1, C0), C1),
        reference=lambda in0, in1, s0, s1, imm2: np.clip(
            in0.astype(np.float32) - in1, s0, s1).astype(np.float32),
    )
    row = dve_ops._CUSTOM_DVE_ROW_BASE + len(dve_ops.OPS)
    dve_ops._SUB_OPCODE_FOR_NAME["SUB_CLIP_ANT"] = row
    shas = {}
    for ver in ("v3", "v4"):
        try:
            uops = lower(spec, ver=ver)
            shas[ver] = DveOpSpec(name="SUB_CLIP_ANT", opcode=row, uops=uops,
                                  rd1_en=True).sha(ver)
        except Exception:
            pass
    op = dve_ops.DveOp("SUB_CLIP_ANT", spec, subdim=False, uops_sha=shas)
    dve_ops.OPS.append(op)
    dve_ops.CUSTOM_DVE_SPECS[op.name] = spec
    _SUBCLIP = op
    return op


# ---------------------------------------------------------------- builder

def _build(Tn=T, taps=()):
    import concourse.bass as bass
    import concourse.mybir as mybir
    from concourse import bacc
    from concourse.tile import TileContext
    from concourse.masks import make_identity

    f32 = mybir.dt.float32
    ALU = mybir.AluOpType
    ACTF = mybir.ActivationFunctionType

    NB = max(1, (Tn * BC) // 512)      # bulk N-chunks over (t, b)
    NBW = (Tn * BC) // NB              # bulk N width (<= 512)
    TB = NBW // BC                     # timesteps per bulk chunk
    NT = Tn // P                       # t-chunks of 128
    CH = 16                            # scan stream chunk (steps)

    subclip = _register_custom_dve()
    nc = bacc.Bacc("TRN2", target_bir_lowering=False)
    bf16 = mybir.dt.bfloat16

    def par(name, shape):
        dt_ = bf16 if name in BF16_PARAMS else f32
        return nc.declare_dram_parameter(name, list(shape), dt_, isOutput=False)

    BF16_PARAMS = {"wrec0T", "tau0avT", "tau0bT", "wrec1T", "tau1avT", "tau1bT"}
    PARAMS = [
        ("x_T", (P, IN // P, Tn * BC)),
        ("win0T", (P, (IN // P) * H)), ("tau0axT", (P, (IN // P) * H)),
        ("tau0avT", (P, HC * H)), ("tau0bT", (P, HC * H)), ("wrec0T", (P, HC * H)),
        ("win1T", (P, HC * H)), ("tau1axT", (P, HC * H)),
        ("tau1avT", (P, HC * H)), ("tau1bT", (P, HC * H)), ("wrec1T", (P, HC * H)),
        ("wkT", (P, HC * H)), ("wvT", (P, HC * H)), ("wqT", (P, HC * H)),
        ("woT", (P, HC * H)), ("p1T", (P, HC * (H // 2))), ("p2T", (P, 2 * OUT)),
        ("b_i0", (P, HC)), ("b_x0", (P, HC)),
        ("b_i1", (P, HC)), ("b_x1", (P, HC)),
        ("b_k", (P, HC)), ("b_q", (P, HC)),
        ("b_o", (P, HC)), ("b_p1", (P, 2)), ("b_p2", (P, 2)),
        ("nrs_i1", (P, HC)), ("nrs_x1", (P, HC)), ("nrs_k", (P, HC)),
        ("gs0bc", (P, HC, BC)), ("gl0bc", (P, HC, BC)), ("tbb0bc", (P, HC, BC)),
        ("gs1bc", (P, HC, BC)), ("gl1bc", (P, HC, BC)), ("tbb1bc", (P, HC, BC)),
        ("rsv_flat", (P, H)), ("bv_flat", (P, H)),
    ]
    PR = {name: par(name, shape) for name, shape in PARAMS}
    out_p = nc.declare_dram_parameter("out", [BC, OUT], f32, isOutput=True)
    tap_h = {}
    SMALL_TAPS = {
        "q": [P, HC, BC], "sc": [NH * BC, Tn], "en": [NH * BC, Tn],
        "oT": [P, HC, BC], "ao": [P, HC, BC], "Vb0": [P, NT * H],
        "KTc": [P, Tn * BC], "xh": [P, HC, BC],
    }
    stp = {}
    for tname in taps:
        if tname in SMALL_TAPS:
            stp[tname] = nc.declare_dram_parameter(
                "tap_" + tname, SMALL_TAPS[tname], f32, isOutput=True)
        else:
            tap_h[tname] = nc.declare_dram_parameter(
                "tap_" + tname, [P, Tn, HC, BC], f32, isOutput=True)

    # Cross-context intermediates (ordered by TileContext exit barriers).
    h0B = nc.dram_tensor("h0B", [P, Tn, HC, BC], f32)
    h1B = nc.dram_tensor("h1B", [P, Tn, HC, BC], f32)
    h1D = nc.dram_tensor("h1D", [P, BC, Tn, HC], f32)

    # ---------------- shared helpers ----------------
    def load(pool, *names):
        out = {}
        for nm in names:
            t_ = pool.tile(list(PR[nm].shape), PR[nm].dtype, tag=nm, name=nm)
            nc.sync.dma_start(out=t_[:], in_=PR[nm][:])
            out[nm] = t_
        return out

    def mmT(ps, w_sb, rhs, nk, hcs=HC, wof=H):
        for hc in range(hcs):
            for kc in range(nk):
                nc.tensor.matmul(
                    ps[:, hc],
                    w_sb[:, kc * wof + hc * P: kc * wof + hc * P + P],
                    rhs[:, kc],
                    start=(kc == 0), stop=(kc == nk - 1))

    def consts(cp):
        ones_col = cp.tile([P, 1], f32, name="ones_col")
        nc.vector.memset(ones_col[:], 1.0)
        ones_row = cp.tile([1, P], f32, name="ones_row")
        nc.vector.memset(ones_row[:], 1.0)
        eps_c = cp.tile([1, 1], f32, name="eps_c")
        nc.vector.memset(eps_c[:], EPS)
        return ones_col, ones_row, eps_c

    def stap(name, ap):
        if name in stp:
            nc.sync.dma_start(out=stp[name][tuple(
                slice(0, s) for s in stp[name].shape)], in_=ap)

    def copy_tap(tc, src, name):
        if name in tap_h:
            with tc.tile_pool(name="tap" + name, bufs=2) as tpp:
                for t in range(Tn):
                    tt = tpp.tile([P, HC, BC], f32, tag="t", name="tt")
                    nc.sync.dma_start(out=tt[:], in_=src[:, t])
                    nc.sync.dma_start(out=tap_h[name][:, t], in_=tt[:])

    def scan_layer(tc, layer, sw, v, g, Isrc, Xsrc, hout, houtD=None):
        L = str(layer)
        wrec, tauav, taub = sw["wrec" + L + "T"], sw["tau" + L + "avT"], sw["tau" + L + "bT"]
        gsbc, glbc, tbbbc = sw["gs" + L + "bc"], sw["gl" + L + "bc"], sw["tbb" + L + "bc"]
        with tc.tile_pool(name="scps" + L, bufs=2, space="PSUM") as pps, \
             tc.tile_pool(name="scwk" + L, bufs=3) as wk, \
             tc.tile_pool(name="scst" + L, bufs=2) as sst:
            ich = xch = None
            for t in range(Tn):
                if t % CH == 0:
                    ich = sst.tile([P, CH, HC, BC], f32, tag="ich", name="ich")
                    xch = sst.tile([P, CH, HC, BC], f32, tag="xch", name="xch")
                    nc.sync.dma_start(out=ich[:], in_=Isrc[:, t:t + CH])
                    nc.sync.dma_start(out=xch[:], in_=Xsrc[:, t:t + CH])
                I0t = ich[:, t % CH]
                X0t = xch[:, t % CH]

                # tau net (uses v at step start)
                vb = wk.tile([P, HC, BC], bf16, tag="vb", name="vb")
                nc.gpsimd.tensor_copy(vb[:], v[:])
                psA = pps.tile([P, HC, BC], f32, tag="psA", name="psA")
                mmT(psA, tauav, vb, HC)
                u2 = wk.tile([P, HC, BC], f32, tag="u2", name="u2")
                nc.vector.tensor_add(u2[:], psA[:], X0t)
                th2 = wk.tile([P, HC, BC], bf16, tag="th2", name="th2")
                nc.scalar.activation(th2[:], u2[:], ACTF.Tanh)
                psB = pps.tile([P, HC, BC], f32, tag="psB", name="psB")
                mmT(psB, taub, th2, HC)
                u3 = wk.tile([P, HC, BC], f32, tag="u3", name="u3")
                nc.vector.tensor_add(u3[:], psB[:], tbbbc[:])
                sig = wk.tile([P, HC, BC], f32, tag="sig", name="sig")
                nc.scalar.activation(sig[:], u3[:], ACTF.Sigmoid)
                tau = wk.tile([P, HC, BC], f32, tag="tau", name="tau")
                nc.vector.tensor_scalar(tau[:], sig[:], MAX_TAU - MIN_TAU,
                                        MIN_TAU, op0=ALU.mult, op1=ALU.add)
                rtau = wk.tile([P, HC, BC], f32, tag="rtau", name="rtau")
                nc.vector.reciprocal_approx_fast(out=rtau[:], in_=tau[:])
                gam = wk.tile([P, HC, BC], f32, tag="gam", name="gam")
                nc.gpsimd.tensor_mul(gam[:], rtau[:], gsbc[:])  # gs pre-scaled DT
                tl = wk.tile([P, HC, BC], f32, tag="tl", name="tl")
                nc.gpsimd.tensor_mul(tl[:], rtau[:], glbc[:])   # gl pre-scaled DT
                cL = wk.tile([P, HC, BC], f32, tag="cL", name="cL")
                nc.gpsimd.tensor_scalar(cL[:], tl[:], -1.0, 1.0,
                                        op0=ALU.mult, op1=ALU.add)

                for u in range(UNFOLDS):
                    th = wk.tile([P, HC, BC], bf16, tag="th", name="th")
                    nc.scalar.activation(th[:], v[:], ACTF.Tanh)
                    if u == 0 and t > 0:
                        th32 = wk.tile([P, HC, BC], f32, tag="th32", name="th32")
                        nc.gpsimd.tensor_copy(th32[:], th[:])
                        nc.sync.dma_start(out=hout[:, t - 1], in_=th32[:])
                        if houtD is not None:
                            for hc in range(HC):
                                nc.sync.dma_start(
                                    out=houtD[:, :, t - 1, hc], in_=th32[:, hc])
                    psI = pps.tile([P, HC, BC], f32, tag="psI", name="psI")
                    mmT(psI, wrec, th, HC)
                    usb = wk.tile([P, HC, BC], f32, tag="usb", name="usb")
                    nc.vector.tensor_add(usb[:], psI[:], I0t)
                    s_ = wk.tile([P, HC, BC], f32, tag="s_", name="s_")
                    nc.scalar.activation(s_[:], usb[:], ACTF.Sigmoid)
                    d_ = wk.tile([P, HC, BC], f32, tag="d_", name="d_")
                    nc.vector.tensor_sub(d_[:], s_[:], g[:])
                    e_ = wk.tile([P, HC, BC], f32, tag="e_", name="e_")
                    nc.vector.scalar_tensor_tensor(e_[:], d_[:], DT / 0.5, rtau[:],
                                                   op0=ALU.mult, op1=ALU.mult)
                    nc.vector.tensor_add(g[:], g[:], e_[:])
                    z_ = wk.tile([P, HC, BC], f32, tag="z_", name="z_")
                    nc.vector.tensor_mul(z_[:], g[:], gam[:])
                    w_ = wk.tile([P, HC, BC], f32, tag="w_", name="w_")
                    nc.gpsimd.tensor_mul(w_[:], v[:], cL[:])
                    tp = wk.tile([P, HC, BC], f32, tag="tp", name="tp")
                    nc.vector.scalar_tensor_tensor(tp[:], v[:], 1.0, z_[:],
                                                   op0=ALU.subtract, op1=ALU.mult)
                    nc.vector._custom_dve(subclip, out=v[:], in0=w_[:], in1=tp[:],
                                          s0=-5.0, s1=5.0)
            thL = sst.tile([P, HC, BC], f32, tag="thL", name="thL")
            nc.scalar.activation(thL[:], v[:], ACTF.Tanh)
            nc.sync.dma_start(out=hout[:, Tn - 1], in_=thL[:])
            if houtD is not None:
                for hc in range(HC):
                    nc.sync.dma_start(out=houtD[:, :, Tn - 1, hc], in_=thL[:, hc])

    def ln_proj(tc, hsrc, targets, cst, m_dst=None, r_dst=None, mrd=None):
        """Per nb chunk: stage h cols, LN stats, then per target emit
        rstd*(h @ W'.T - m*RS') + bias' via dst callback."""
        ones_col, ones_row, eps_c = cst
        with tc.tile_pool(name="lnst", bufs=2) as lst, \
             tc.tile_pool(name="lnsm", bufs=2) as lsm, \
             tc.tile_pool(name="lnps", bufs=2, space="PSUM") as lps, \
             tc.tile_pool(name="lnbc", bufs=1, space="PSUM") as lbc, \
             tc.tile_pool(name="lnqs", bufs=1, space="PSUM") as lqs:
            for nb in range(NB):
                t0, t1 = nb * TB, (nb + 1) * TB
                hcs = []
                for hc in range(HC):
                    hsb = lst.tile([P, NBW], f32, tag=f"h{hc}", name="hsb")
                    nc.sync.dma_start(
                        out=hsb[:].rearrange("p (t b) -> p t b", t=TB, b=BC),
                        in_=hsrc[:, t0:t1, hc, :])
                    hcs.append(hsb)
                psS = lqs.tile([1, NBW], f32, tag="psS", name="psS")
                for hc in range(HC):
                    nc.tensor.matmul(psS[:], ones_col[:], hcs[hc][:],
                                     start=(hc == 0), stop=(hc == HC - 1))
                psQ = lqs.tile([1, NBW], f32, tag="psQ", name="psQ")
                for hc in range(HC):
                    sq = lst.tile([P, NBW], f32, tag="sq", name="sq")
                    nc.scalar.activation(sq[:], hcs[hc][:], ACTF.Square)
                    nc.tensor.matmul(psQ[:], ones_col[:], sq[:],
                                     start=(hc == 0), stop=(hc == HC - 1))
                if m_dst is None:
                    m_ = lsm.tile([1, NBW], f32, tag="m_", name="m_")[:]
                    r_ = lsm.tile([1, NBW], f32, tag="r_", name="r_")[:]
                else:
                    m_ = m_dst[:, nb * NBW:(nb + 1) * NBW]
                    r_ = r_dst[:, nb * NBW:(nb + 1) * NBW]
                nc.scalar.activation(m_, psS[:], ACTF.Copy, scale=1.0 / H)
                msq = lsm.tile([1, NBW], f32, tag="msq", name="msq")
                nc.scalar.activation(msq[:], psQ[:], ACTF.Copy, scale=1.0 / H)
                mm_ = lsm.tile([1, NBW], f32, tag="mm_", name="mm_")
                nc.vector.tensor_mul(mm_[:], m_, m_)
                var = lsm.tile([1, NBW], f32, tag="var", name="var")
                nc.vector.tensor_sub(var[:], msq[:], mm_[:])
                std = lsm.tile([1, NBW], f32, tag="std", name="std")
                nc.scalar.activation(std[:], var[:], ACTF.Sqrt, bias=eps_c[:])
                nc.vector.reciprocal_approx_fast(out=r_, in_=std[:])
                psM = lbc.tile([P, NBW], f32, tag="psM", name="psM")
                nc.tensor.matmul(psM[:], ones_row[:], m_, start=True, stop=True)
                mB = lst.tile([P, NBW], f32, tag="mB", name="mB")
                nc.scalar.copy(mB[:], psM[:])
                psR = lbc.tile([P, NBW], f32, tag="psR", name="psR")
                nc.tensor.matmul(psR[:], ones_row[:], r_, start=True, stop=True)
                rB = lst.tile([P, NBW], f32, tag="rB", name="rB")
                nc.scalar.copy(rB[:], psR[:])
                for tg in targets:
                    w_sb, nrs, bias = tg["w"], tg["nrs"], tg["bias"]
                    for hc in range(HC):
                        psP = lps.tile([P, NBW], f32, tag="psP", name="psP")
                        for kc in range(HC):
                            nc.tensor.matmul(
                                psP[:],
                                w_sb[:, kc * H + hc * P: kc * H + hc * P + P],
                                hcs[kc][:],
                                start=(kc == 0), stop=(kc == HC - 1))
                        t2 = lst.tile([P, NBW], f32, tag="t2", name="t2")
                        nc.vector.scalar_tensor_tensor(
                            t2[:], mB[:], nrs[:, hc:hc + 1], psP[:],
                            op0=ALU.mult, op1=ALU.add)
                        f_ = lst.tile([P, NBW], f32, tag="f_", name="f_")
                        nc.vector.tensor_mul(f_[:], t2[:], rB[:])
                        tg["dst"](nb, hc, f_, bias, t0, t1)
                if mrd is not None:
                    nc.sync.dma_start(out=mrd[0:1, nb * NBW:(nb + 1) * NBW], in_=m_)
                    nc.sync.dma_start(out=mrd[1:2, nb * NBW:(nb + 1) * NBW], in_=r_)

    def dst_dram(dram, stg_pool):
        def f(nb, hc, f_, bias, t0, t1):
            stt = stg_pool.tile([P, NBW], f32, tag="stg", name="stt")
            nc.scalar.activation(stt[:], f_[:], ACTF.Identity,
                                 bias=bias[:, hc:hc + 1])
            nc.sync.dma_start(
                out=dram[:, t0:t1, hc, :],
                in_=stt[:].rearrange("p (t b) -> p t b", t=TB, b=BC))
        return f

    # ============ CONTEXT 1: P1 bulk input projections + layer-0 scan ========
    with TileContext(nc) as tc:
        with tc.tile_pool(name="c1dram", bufs=1, space="DRAM") as dp1, \
             tc.tile_pool(name="c1state", bufs=1) as sp1:
            I0d = dp1.tile([P, Tn, HC, BC], f32, tag="I0d", name="I0d")
            X0d = dp1.tile([P, Tn, HC, BC], f32, tag="X0d", name="X0d")
            v0 = sp1.tile([P, HC, BC], f32, name="v0")
            g0 = sp1.tile([P, HC, BC], f32, name="g0")
            nc.vector.memset(v0[:], 0.0)
            nc.vector.memset(g0[:], 0.0)

            with tc.tile_pool(name="p1w", bufs=1) as p1w, \
                 tc.tile_pool(name="p1st", bufs=3) as stg, \
                 tc.tile_pool(name="p1ps", bufs=2, space="PSUM") as pp:
                wb = load(p1w, "win0T", "tau0axT", "b_i0", "b_x0")
                xsb = p1w.tile([P, IN // P, Tn * BC], f32, name="xsb")
                nc.sync.dma_start(out=xsb[:], in_=PR["x_T"][:])
                nkx = IN // P
                for (wnm, bnm, dst) in [("win0T", "b_i0", I0d),
                                        ("tau0axT", "b_x0", X0d)]:
                    for hc in range(HC):
                        for nb in range(NB):
                            ps = pp.tile([P, NBW], f32, tag="ps", name="ps")
                            for kc in range(nkx):
                                nc.tensor.matmul(
                                    ps[:],
                                    wb[wnm][:, kc * H + hc * P: kc * H + hc * P + P],
                                    xsb[:, kc, nb * NBW:(nb + 1) * NBW],
                                    start=(kc == 0), stop=(kc == nkx - 1))
                            stt = stg.tile([P, NBW], f32, tag="st", name="stt")
                            nc.scalar.activation(stt[:], ps[:], ACTF.Identity,
                                                 bias=wb[bnm][:, hc:hc + 1])
                            t0, t1 = nb * TB, (nb + 1) * TB
                            nc.sync.dma_start(
                                out=dst[:, t0:t1, hc, :],
                                in_=stt[:].rearrange("p (t b) -> p t b",
                                                     t=TB, b=BC))

            with tc.tile_pool(name="scanw0", bufs=1) as scw:
                sw = load(scw, "wrec0T", "tau0avT", "tau0bT",
                          "gs0bc", "gl0bc", "tbb0bc")
                scan_layer(tc, 0, sw, v0, g0, I0d, X0d, h0B)
            copy_tap(tc, h0B, "h0")

    # ============ CONTEXT 2: P3 LN0-folded projections + layer-1 scan ========
    with TileContext(nc) as tc:
        with tc.tile_pool(name="c2dram", bufs=1, space="DRAM") as dp2, \
             tc.tile_pool(name="c2const", bufs=1) as cp2, \
             tc.tile_pool(name="c2state", bufs=1) as sp2:
            I1d = dp2.tile([P, Tn, HC, BC], f32, tag="I1d", name="I1d")
            X1d = dp2.tile([P, Tn, HC, BC], f32, tag="X1d", name="X1d")
            cst2 = consts(cp2)
            v1 = sp2.tile([P, HC, BC], f32, name="v1")
            g1 = sp2.tile([P, HC, BC], f32, name="g1")
            nc.vector.memset(v1[:], 0.0)
            nc.vector.memset(g1[:], 0.0)

            with tc.tile_pool(name="p3w", bufs=1) as p3w, \
                 tc.tile_pool(name="p3stg", bufs=3) as p3stg:
                w3 = load(p3w, "win1T", "tau1axT", "nrs_i1", "nrs_x1",
                          "b_i1", "b_x1")
                ln_proj(tc, h0B, [
                    {"w": w3["win1T"], "nrs": w3["nrs_i1"], "bias": w3["b_i1"],
                     "dst": dst_dram(I1d, p3stg)},
                    {"w": w3["tau1axT"], "nrs": w3["nrs_x1"], "bias": w3["b_x1"],
                     "dst": dst_dram(X1d, p3stg)},
                ], cst2)
            copy_tap(tc, I1d, "I1")

            with tc.tile_pool(name="scanw1", bufs=1) as scw:
                sw = load(scw, "wrec1T", "tau1avT", "tau1bT",
                          "gs1bc", "gl1bc", "tbb1bc")
                scan_layer(tc, 1, sw, v1, g1, I1d, X1d, h1B, houtD=h1D)
            copy_tap(tc, h1B, "h1")

    # ============ CONTEXT 3: attention (last timestep) + head ================
    with TileContext(nc) as tc:
        with tc.tile_pool(name="c3dram", bufs=1, space="DRAM") as dp3, \
             tc.tile_pool(name="c3const", bufs=1) as cp3, \
             tc.tile_pool(name="atw", bufs=1) as atw:
            mrd = dp3.tile([2, Tn * BC], f32, tag="mrd", name="mrd")
            cst3 = consts(cp3)
            ones_col, ones_row, eps_c = cst3
            ident = cp3.tile([NH * BC, NH * BC], f32, name="ident")
            make_identity(nc, ident[:])
            m1_sb = cp3.tile([1, Tn * BC], f32, name="m1_sb")
            r1_sb = cp3.tile([1, Tn * BC], f32, name="r1_sb")

            wa = load(atw, "wkT", "wvT", "wqT", "woT", "p1T", "p2T",
                      "b_k", "b_q", "b_o", "b_p1", "b_p2", "nrs_k",
                      "rsv_flat", "bv_flat")
            KT = atw.tile([P, HC * Tn * BC], f32, name="KT")

            def dst_K(nb, hc, f_, bias, t0, t1):
                nc.scalar.activation(
                    KT[:, hc * Tn * BC + nb * NBW: hc * Tn * BC + (nb + 1) * NBW],
                    f_[:], ACTF.Identity, bias=bias[:, hc:hc + 1])

            ln_proj(tc, h1B,
                    [{"w": wa["wkT"], "nrs": wa["nrs_k"], "bias": wa["b_k"],
                      "dst": dst_K}],
                    cst3, m_dst=m1_sb, r_dst=r1_sb, mrd=mrd)
            stap("KTc", KT[:, 0:Tn * BC])

            with tc.tile_pool(name="vstats", bufs=1) as vsp:
                m1T = vsp.tile([P, NT, BC], f32, name="m1T")
                r1T = vsp.tile([P, NT, BC], f32, name="r1T")
                nc.sync.dma_start(
                    out=m1T[:],
                    in_=mrd[0:1, :].rearrange("o (tc p b) -> (o p) tc b",
                                              tc=NT, p=P, b=BC))
                nc.sync.dma_start(
                    out=r1T[:],
                    in_=mrd[1:2, :].rearrange("o (tc p b) -> (o p) tc b",
                                              tc=NT, p=P, b=BC))

                # ---- q_last ----
                qT = vsp.tile([P, HC, BC], f32, name="qT")
                with tc.tile_pool(name="qps", bufs=1, space="PSUM") as qps, \
                     tc.tile_pool(name="qwk", bufs=2) as qwk, \
                     tc.tile_pool(name="qc", bufs=1) as qc:
                    hL = qc.tile([P, HC, BC], f32, name="hL")
                    nc.sync.dma_start(out=hL[:], in_=h1B[:, Tn - 1])
                    psb = qps.tile([P, BC], f32, tag="psb", name="psb")
                    nc.tensor.matmul(psb[:], ones_row[:],
                                     m1_sb[:, (Tn - 1) * BC: Tn * BC],
                                     start=True, stop=True)
                    mL = qc.tile([P, BC], f32, name="mL")
                    nc.scalar.copy(mL[:], psb[:])
                    psb2 = qps.tile([P, BC], f32, tag="psb2", name="psb2")
                    nc.tensor.matmul(psb2[:], ones_row[:],
                                     r1_sb[:, (Tn - 1) * BC: Tn * BC],
                                     start=True, stop=True)
                    rL = qc.tile([P, BC], f32, name="rL")
                    nc.scalar.copy(rL[:], psb2[:])
                    xh = qc.tile([P, HC, BC], f32, name="xh")
                    for hc in range(HC):
                        tt = qwk.tile([P, BC], f32, tag="xh1", name="tt")
                        nc.vector.tensor_sub(tt[:], hL[:, hc], mL[:])
                        nc.vector.tensor_mul(xh[:, hc], tt[:], rL[:])
                    psq = qps.tile([P, HC, BC], f32, tag="psq", name="psq")
                    mmT(psq, wa["wqT"], xh, HC)
                    for hc in range(HC):
                        nc.scalar.activation(qT[:, hc], psq[:, hc], ACTF.Identity,
                                             bias=wa["b_q"][:, hc:hc + 1])
                    stap("xh", xh[:])
                stap("q", qT[:])

                # ---- scores + softmax ----
                sc = vsp.tile([NH * BC, Tn], f32, name="sc")
                with tc.tile_pool(name="scops", bufs=2, space="PSUM") as sps, \
                     tc.tile_pool(name="scowk", bufs=3) as swk:
                    for b in range(BC):
                        for h in range(NH):
                            pss = sps.tile([1, Tn], f32, tag="pss", name="pss")
                            nc.tensor.matmul(
                                pss[:],
                                qT[:, h, b:b + 1],
                                KT[:, h * Tn * BC + b: (h + 1) * Tn * BC: BC],
                                start=True, stop=True)
                            srow = swk.tile([1, Tn], f32, tag="srow", name="srow")
                            nc.scalar.copy(srow[:], pss[:])
                            nc.sync.dma_start(
                                out=sc[b * NH + h: b * NH + h + 1, :], in_=srow[:])
                stap("sc", sc[:])
                mx = vsp.tile([NH * BC, 1], f32, name="mx")
                nc.vector.tensor_reduce(mx[:], sc[:], axis=mybir.AxisListType.X,
                                        op=ALU.max)
                nmx = vsp.tile([NH * BC, 1], f32, name="nmx")
                nc.vector.tensor_scalar(nmx[:], mx[:], -1.0, None, op0=ALU.mult)
                ex = vsp.tile([NH * BC, Tn], f32, name="ex")
                sm = vsp.tile([NH * BC, 1], f32, name="sm")
                nc.scalar.activation(ex[:], sc[:], ACTF.Exp, bias=nmx[:],
                                     accum_out=sm[:])
                rsm = vsp.tile([NH * BC, 1], f32, name="rsm")
                nc.vector.reciprocal_approx_fast(out=rsm[:], in_=sm[:])
                en = vsp.tile([NH * BC, Tn], f32, name="en")
                nc.vector.tensor_scalar(en[:], ex[:], rsm[:], None, op0=ALU.mult)
                stap("en", en[:])
                eT = []
                with tc.tile_pool(name="etps", bufs=2, space="PSUM") as eps_p:
                    for kc in range(NT):
                        pse = eps_p.tile([P, NH * BC], f32, tag="pse", name="pse")
                        nc.tensor.transpose(pse[:], en[:, kc * P:(kc + 1) * P],
                                            ident[:])
                        esb = vsp.tile([P, NH * BC], f32, name=f"eT{kc}",
                                       tag=f"eT{kc}")
                        nc.scalar.copy(esb[:], pse[:])
                        eT.append(esb)

                # ---- V projection (per example) + o ----
                psO_sb = vsp.tile([P, HC, BC], f32, name="psO_sb")
                with tc.tile_pool(name="vps", bufs=2, space="PSUM") as vps, \
                     tc.tile_pool(name="vwk", bufs=2) as vwk, \
                     tc.tile_pool(name="ops", bufs=1, space="PSUM") as ops_:
                    psO = ops_.tile([P, HC, BC], f32, tag="psO", name="psO")
                    for b in range(BC):
                        hb = vwk.tile([P, Tn, HC], f32, tag="hb", name="hb")
                        nc.sync.dma_start(out=hb[:], in_=h1D[:, b])
                        Vb = vwk.tile([P, NT * H], f32, tag="Vb", name="Vb")
                        for tcc in range(NT):
                            psV = vps.tile([P, H], f32, tag="psV", name="psV")
                            for kc in range(HC):
                                nc.tensor.matmul(
                                    psV[:],
                                    hb[:, tcc * P:(tcc + 1) * P, kc],
                                    wa["wvT"][:, kc * H:(kc + 1) * H],
                                    start=(kc == 0), stop=(kc == HC - 1))
                            t2 = vwk.tile([P, H], f32, tag="t2v", name="t2")
                            nc.vector.scalar_tensor_tensor(
                                t2[:], wa["rsv_flat"][:], m1T[:, tcc, b:b + 1],
                                psV[:], op0=ALU.mult, op1=ALU.add)
                            f_ = vwk.tile([P, H], f32, tag="f_v", name="f_")
                            nc.vector.tensor_scalar(
                                f_[:], t2[:], r1T[:, tcc, b:b + 1], None,
                                op0=ALU.mult)
                            nc.vector.tensor_add(
                                Vb[:, tcc * H:(tcc + 1) * H], f_[:],
                                wa["bv_flat"][:])
                        if b == 0:
                            stap("Vb0", Vb[:])
                        for h in range(NH):
                            for kc in range(NT):
                                nc.tensor.matmul(
                                    psO[:, h, b:b + 1],
                                    Vb[:, kc * H + h * HD: kc * H + (h + 1) * HD],
                                    eT[kc][:, b * NH + h: b * NH + h + 1],
                                    start=(kc == 0), stop=(kc == NT - 1))
                    nc.scalar.copy(psO_sb[:], psO[:])
                    stap("oT", psO_sb[:])

                # ---- head ----
                with tc.tile_pool(name="hps", bufs=1, space="PSUM") as hps, \
                     tc.tile_pool(name="hc_", bufs=1) as hcp:
                    psAO = hps.tile([P, HC, BC], f32, tag="psAO", name="psAO")
                    mmT(psAO, wa["woT"], psO_sb, HC)
                    ao = hcp.tile([P, HC, BC], f32, name="ao")
                    for hc in range(HC):
                        nc.scalar.activation(ao[:, hc], psAO[:, hc], ACTF.Identity,
                                             bias=wa["b_o"][:, hc:hc + 1])
                    stap("ao", ao[:])
                    psP1 = hps.tile([P, 2, BC], f32, tag="psP1", name="psP1")
                    for hc in range(2):
                        for kc in range(HC):
                            nc.tensor.matmul(
                                psP1[:, hc],
                                wa["p1T"][:, kc * (H // 2) + hc * P:
                                          kc * (H // 2) + hc * P + P],
                                ao[:, kc],
                                start=(kc == 0), stop=(kc == HC - 1))
                    h1_ = hcp.tile([P, 2, BC], f32, name="h1_")
                    for hc in range(2):
                        nc.scalar.activation(h1_[:, hc], psP1[:, hc], ACTF.Relu,
                                             bias=wa["b_p1"][:, hc:hc + 1])
                    psP2 = hps.tile([P, 2, BC], f32, tag="psP2", name="psP2")
                    for hc in range(2):
                        for kc in range(2):
                            nc.tensor.matmul(
                                psP2[:, hc],
                                wa["p2T"][:, kc * OUT + hc * P:
                                          kc * OUT + hc * P + P],
                                h1_[:, kc],
                                start=(kc == 0), stop=(kc == 1))
                    outT = hcp.tile([P, 2, BC], f32, name="outT")
                    for hc in range(2):
                        nc.scalar.activation(outT[:, hc], psP2[:, hc],
                                             ACTF.Identity,
                                             bias=wa["b_p2"][:, hc:hc + 1])
                    for c in range(2):
                        nc.sync.dma_start(
                            out=out_p[:, c * P:(c + 1) * P].rearrange("b p -> p b"),
                            in_=outT[:, c])

    nc.finalize()
    return nc


# ---------------------------------------------------------------- host driver

def _prep_inputs(inputs, Tn=T):
    d = {k: np.asarray(v, np.float32) for k, v in inputs.items()}
    sqh = np.float32(1.0 / np.sqrt(HD))

    wi1, bi1, rs_i1 = _fold3(d["Win1_w"], d["Win1_b"], d["ln0_w"], d["ln0_b"])
    wx1, bx1, rs_x1 = _fold3(d["tau1a_w"][:, :H], d["tau1a_b"], d["ln0_w"], d["ln0_b"])
    ab = d["attn_in_b"]
    wq, bq, _ = _fold3(d["attn_in_w"][0:H] * sqh, ab[0:H] * sqh, d["ln1_w"], d["ln1_b"])
    wk, bk, rs_k = _fold3(d["attn_in_w"][H:2 * H], ab[H:2 * H], d["ln1_w"], d["ln1_b"])
    wv, bv, rs_v = _fold3(d["attn_in_w"][2 * H:3 * H], ab[2 * H:3 * H],
                          d["ln1_w"], d["ln1_b"])

    common = {
        "win0T": _wT(d["Win0_w"]), "tau0axT": _wT(d["tau0a_w"][:, :IN]),
        "tau0avT": _wT(d["tau0a_w"][:, IN:]), "tau0bT": _wT(d["tau0b_w"]),
        "wrec0T": _wT(d["Wrec0_w"]),
        "win1T": _wT(wi1), "tau1axT": _wT(wx1),
        "tau1avT": _wT(d["tau1a_w"][:, H:]), "tau1bT": _wT(d["tau1b_w"]),
        "wrec1T": _wT(d["Wrec1_w"]),
        "wkT": _wT(wk), "wvT": _wT(wv), "wqT": _wT(wq),
        "woT": _wT(d["attn_out_w"]), "p1T": _wT(d["p1_w"]), "p2T": _wT(d["p2_w"]),
        "b_i0": _perH(d["Win0_b"]), "b_x0": _perH(d["tau0a_b"]),
        "b_i1": _perH(bi1), "b_x1": _perH(bx1),
        "b_k": _perH(bk), "b_q": _perH(bq),
        "b_o": _perH(d["attn_out_b"]),
        "b_p1": _perH(d["p1_b"]), "b_p2": _perH(d["p2_b"]),
        "nrs_i1": _perH(-rs_i1), "nrs_x1": _perH(-rs_x1), "nrs_k": _perH(-rs_k),
        "gs0bc": _bcast(DT * d["gsyn0"]), "gl0bc": _bcast(DT * d["gleak0"]),
        "tbb0bc": _bcast(d["tau0b_b"]),
        "gs1bc": _bcast(DT * d["gsyn1"]), "gl1bc": _bcast(DT * d["gleak1"]),
        "tbb1bc": _bcast(d["tau1b_b"]),
        "rsv_flat": np.ascontiguousarray(
            np.broadcast_to((-rs_v)[None, :], (P, H))).astype(np.float32),
        "bv_flat": np.ascontiguousarray(
            np.broadcast_to(bv[None, :], (P, H))).astype(np.float32),
    }
    import ml_dtypes
    for nm in ("wrec0T", "tau0avT", "tau0bT", "wrec1T", "tau1avT", "tau1bT"):
        common[nm] = common[nm].astype(ml_dtypes.bfloat16)
    x = d["inputs"][:, :Tn]
    in_maps = []
    for c in range(NCORES):
        m = dict(common)
        m["x_T"] = _xT(x[c * BC:(c + 1) * BC])
        in_maps.append(m)
    return in_maps


def _run(inputs, trace=False, Tn=T, taps=()):
    from concourse.bass_utils import run_bass_kernel_spmd
    key = (Tn, tuple(taps))
    if key not in _CACHE:
        _CACHE[key] = _build(Tn, taps)
    nc = _CACHE[key]
    in_maps = _prep_inputs(inputs, Tn)
    res = run_bass_kernel_spmd(nc, in_maps, list(range(NCORES)), trace=trace)
    outs = [r["out"] for r in res.results]
    full = np.concatenate(outs, axis=0).astype(np.float32)
    return full, res


def kernel(**inputs):
    out, _ = _run(inputs, trace=False)
    return out
